# revision 1
# baseline (speedup 1.0000x reference)
"""Trainium2 Bass kernel for BitNet multi-head attention (nn_MultiHeadAttention_62294205661880).

Sharding: 8 cores = 2 batches x 4 head-groups (4 heads each).  Each core
computes qkv projection, RoPE, causal attention and a column-parallel slice
of the output projection for its (batch, head-group); the host sums the 4
partial out-projections per batch (the tensor-parallel all-reduce done
host-side, since the contract gathers to host anyway).

BitNet quantization is folded on the host: weights are uploaded as exact
ternary {-1,0,+1} bf16 matrices; scale_qkv^2/sqrt(dh) is folded into the
softmax exp() scale and scale_qkv*scale_out into a final host-side scalar.

Device layout trick: everything is computed transposed.  Q_T/K_T come out of
the projection as [dh, S]; scores are computed as s_T[k, q]; the softmax
denominator sums over the partition (key) dim via an all-ones stationary
matmul (which also replicates the sums across partitions for free); AV
produces out_T[dh, q] which feeds the output projection directly.  No
on-device transposes at all.  Softmax skips the max-subtraction: scores are
bounded (~+-2) because the BitNet weight scale is tiny, so exp() is safe.
"""

import sys
import types

import numpy as np
import ml_dtypes

import concourse.bass as bass
import concourse.mybir as mybir
import concourse.tile as tile
from concourse import bacc
from concourse.bass_utils import run_bass_kernel_spmd

D_MODEL = 2048
N_HEADS = 16
D_HEAD = 128
SEQ = 2048
BATCH = 2
ROPE_BASE = 10000.0

N_CORES = 8
HPC = 4  # heads per core
R_LOCAL = HPC * D_HEAD  # 512 local q (or k, or v) rows per core
MO = D_MODEL // 128  # 16 contraction blocks
NKI = SEQ // 128  # 16 key blocks
NQC = SEQ // 512  # 4 query chunks of 512
NSB = SEQ // 128  # 16 seq blocks (v / proj)

BF16 = mybir.dt.bfloat16
F32 = mybir.dt.float32
NPBF16 = ml_dtypes.bfloat16
NPFP8 = ml_dtypes.float8_e4m3
FP8 = mybir.dt.float8e4

LAST_RESULT = None  # BassKernelResults of the most recent run (for test.py)
_PROG_CACHE = {}
PROFILE = False  # test.py sets True to capture an NTFF profile / HW exec time


def _enable_profiling() -> bool:
    """Install the axon NTFF profile hook glue if the image lacks
    ``antenv.axon_hooks`` (boot degrades silently without it), and skip
    the artifact upload (no bucket access in this container)."""
    try:
        from antenv.axon_hooks import get_axon_ntff_profile_hook  # noqa: F401

        ok = get_axon_ntff_profile_hook() is not None
    except ImportError:
        ok = False
        import antenv

        mod = types.ModuleType("antenv.axon_hooks")
        mod._hook = None
        mod.set_axon_ntff_profile_hook = lambda h: setattr(mod, "_hook", h)
        mod.get_axon_ntff_profile_hook = lambda: mod._hook
        sys.modules["antenv.axon_hooks"] = mod
        antenv.axon_hooks = mod
        try:
            from trn_agent_boot.trn_boot import _ntff_profile_via_ctypes

            hook = _ntff_profile_via_ctypes("/opt/axon/libaxon_pjrt.so")
            if hook is not None:
                mod._hook = hook
                ok = True
        except Exception as e:  # profiling is best-effort
            print(f"ntff profile hook install failed: {e}", file=sys.stderr)
    if ok:
        import concourse.bass_utils as _bu

        _bu.upload_artifacts = lambda tmpdir: tmpdir
    return ok


def _build_program(causal: bool, exp_scale: float) -> bass.Bass:
    nc = bacc.Bacc(None)
    S = SEQ

    xT_d = nc.dram_tensor("xT", [D_MODEL, S], BF16, kind="ExternalInput")
    wqT_d = nc.dram_tensor("wqT", [D_MODEL, R_LOCAL], FP8, kind="ExternalInput")
    wkT_d = nc.dram_tensor("wkT", [D_MODEL, R_LOCAL], FP8, kind="ExternalInput")
    wvT_d = nc.dram_tensor("wvT", [D_MODEL, R_LOCAL], FP8, kind="ExternalInput")
    woT_d = nc.dram_tensor("woT", [R_LOCAL, D_MODEL], BF16, kind="ExternalInput")
    # cos rows 0:64, sin rows 64:128
    cs_d = nc.dram_tensor("cossinT", [128, S], BF16, kind="ExternalInput")
    # swapped: sin rows 0:64, cos rows 64:128 (keeps TensorTensor base partitions equal)
    sc_d = nc.dram_tensor("sincosT", [128, S], BF16, kind="ExternalInput")
    if causal:
        # 16 transposed diagonal 128x128 mask blocks, side by side
        maskd_d = nc.dram_tensor("maskd", [128, S], BF16, kind="ExternalInput")
    else:
        maskf_d = nc.dram_tensor("maskf", [S, S], BF16, kind="ExternalInput")
    out_d = nc.dram_tensor("out", [S, D_MODEL], BF16, kind="ExternalOutput")

    xT_v = xT_d[:].rearrange("(mo p) s -> p mo s", p=128)
    wqT_v = wqT_d[:].rearrange("(mo p) r -> p mo r", p=128)
    wkT_v = wkT_d[:].rearrange("(mo p) r -> p mo r", p=128)
    wvT_v = wvT_d[:].rearrange("(mo p) r -> p mo r", p=128)
    woT_v = woT_d[:].rearrange("(h p) o -> p h o", p=128)
    if not causal:
        maskf_v = maskf_d[:].rearrange("(ko p) q -> p ko q", p=128)

    with tile.TileContext(nc) as tc:
        with tc.tile_pool(name="pers", bufs=1) as pers:
            # ---- persistent SBUF tensors (live across both phases) ----
            q_rot = pers.tile([128, HPC, S], BF16, tag="qrot")
            k_rot = pers.tile([128, HPC, S], BF16, tag="krot")
            v_sb = pers.tile([128, NKI, R_LOCAL], BF16, tag="vsb")
            aoT = pers.tile([128, HPC, S], BF16, tag="aoT")
            ones_t = pers.tile([128, 128], BF16, tag="ones")
            warm = pers.tile([128, 1], BF16, tag="warm")
            if causal:
                maskd = pers.tile([128, S], BF16, tag="maskd")
            nc.vector.memset(ones_t[:, :], 1.0)
            # load the exp table set first so no ACT table switch happens
            # mid-kernel (Copy lives in every set).
            nc.scalar.activation(
                warm[:, :], ones_t[:, 0:1], mybir.ActivationFunctionType.Exp
            )

            # ================= phase A: QKV projection + RoPE =================
            with (
                tc.tile_pool(name="xtp", bufs=1) as xtp,
                tc.tile_pool(name="wp", bufs=1) as wp,
                tc.tile_pool(name="raw", bufs=2) as rawp,
                tc.tile_pool(name="w8", bufs=6) as w8p,
                tc.tile_pool(name="tmp", bufs=2) as tmpp,
                tc.tile_pool(name="psA", bufs=2, space="PSUM") as psA,
            ):
                xt = xtp.tile([128, MO, S], BF16, tag="xt")
                wq = wp.tile([128, MO, R_LOCAL], BF16, tag="wq")
                wk = wp.tile([128, MO, R_LOCAL], BF16, tag="wk")
                wv = wp.tile([128, MO, R_LOCAL], BF16, tag="wv")
                cs_t = wp.tile([128, S], BF16, tag="cs")
                sc_t = wp.tile([128, S], BF16, tag="sc")

                def load_w8(dst, view, mo):
                    st = w8p.tile([128, R_LOCAL], FP8, tag="w8")
                    nc.sync.dma_start(out=st[:, :], in_=view[:, mo, :])
                    nc.vector.tensor_copy(dst[:, mo, :], st[:, :])

                for mo in range(MO):
                    load_w8(wq, wqT_v, mo)
                    load_w8(wk, wkT_v, mo)
                    if mo < 4:
                        nc.sync.dma_start(
                            out=xt[:, mo, 0:1024], in_=xT_v[:, mo, 0:1024]
                        )
                        nc.sync.dma_start(
                            out=xt[:, mo, 1024:2048], in_=xT_v[:, mo, 1024:2048]
                        )
                    else:
                        nc.sync.dma_start(out=xt[:, mo, :], in_=xT_v[:, mo, :])
                nc.sync.dma_start(out=cs_t[:, :], in_=cs_d[:, :])
                nc.sync.dma_start(out=sc_t[:, :], in_=sc_d[:, :])
                if causal:
                    nc.sync.dma_start(out=maskd[:, :], in_=maskd_d[:, :])
                for mo in range(MO):
                    load_w8(wv, wvT_v, mo)

                def rope(dst, raw):
                    """NeoX rotary: rows 0:64 = t*c - b*s ; rows 64:128 = t*s + b*c."""
                    ta = tmpp.tile([64, S], BF16, tag="tmp")
                    tb = tmpp.tile([64, S], BF16, tag="tmp")
                    nc.vector.tensor_mul(ta[:, :], raw[0:64, :], cs_t[0:64, :])
                    nc.vector.tensor_mul(tb[:, :], raw[64:128, :], cs_t[64:128, :])
                    nc.vector.tensor_sub(dst[0:64, :], ta[:, :], tb[:, :])
                    tc2 = tmpp.tile([64, S], BF16, tag="tmp")
                    td = tmpp.tile([64, S], BF16, tag="tmp")
                    nc.vector.tensor_mul(tc2[:, :], raw[0:64, :], sc_t[0:64, :])
                    nc.vector.tensor_mul(td[:, :], raw[64:128, :], sc_t[64:128, :])
                    nc.vector.tensor_add(dst[64:128, :], tc2[:, :], td[:, :])

                # head 0 q/k with the m-loop OUTER so the matmuls consume
                # xt m-blocks as the DMAs land (startup overlap).
                qp0 = psA.tile([128, S], F32, tag="psA")
                kp0 = psA.tile([128, S], F32, tag="psA")
                for m in range(MO):
                    for c4 in range(4):
                        nc.tensor.matmul(
                            qp0[:, c4 * 512 : (c4 + 1) * 512],
                            wq[:, m, 0:128],
                            xt[:, m, c4 * 512 : (c4 + 1) * 512],
                            start=(m == 0),
                            stop=(m == MO - 1),
                        )
                        nc.tensor.matmul(
                            kp0[:, c4 * 512 : (c4 + 1) * 512],
                            wk[:, m, 0:128],
                            xt[:, m, c4 * 512 : (c4 + 1) * 512],
                            start=(m == 0),
                            stop=(m == MO - 1),
                        )
                q_raw = rawp.tile([128, S], BF16, tag="raw")
                nc.scalar.copy(q_raw[:, :], qp0[:, :])
                rope(q_rot[:, 0, :], q_raw)
                k_raw = rawp.tile([128, S], BF16, tag="raw")
                nc.scalar.copy(k_raw[:, :], kp0[:, :])
                rope(k_rot[:, 0, :], k_raw)

                def project(dst_raw, w_sb, h):
                    """q/k head projection -> bf16 raw [128, S] (xt resident)."""
                    ps = psA.tile([128, S], F32, tag="psA")
                    for c4 in range(4):
                        for m in range(MO):
                            nc.tensor.matmul(
                                ps[:, c4 * 512 : (c4 + 1) * 512],
                                w_sb[:, m, h * 128 : (h + 1) * 128],
                                xt[:, m, c4 * 512 : (c4 + 1) * 512],
                                start=(m == 0),
                                stop=(m == MO - 1),
                            )
                    nc.scalar.copy(dst_raw[:, :], ps[:, :])

                for h in range(1, HPC):
                    q_raw = rawp.tile([128, S], BF16, tag="raw")
                    project(q_raw, wq, h)
                    rope(q_rot[:, h, :], q_raw)
                    k_raw = rawp.tile([128, S], BF16, tag="raw")
                    project(k_raw, wk, h)
                    rope(k_rot[:, h, :], k_raw)

                # V projection (natural layout [s, r]); 4 seq blocks per psum
                for sb4 in range(NSB // 4):
                    ps = psA.tile([128, S], F32, tag="psA")
                    for part in range(4):
                        sb = sb4 * 4 + part
                        for m in range(MO):
                            nc.tensor.matmul(
                                ps[:, part * 512 : part * 512 + 512],
                                xt[:, m, sb * 128 : (sb + 1) * 128],
                                wv[:, m, :],
                                start=(m == 0),
                                stop=(m == MO - 1),
                            )
                    nc.scalar.copy(v_sb[:, sb4 * 4 : sb4 * 4 + 4, :], ps[:, :])

            # ================= phase B: attention + out-projection =============
            with (
                tc.tile_pool(name="wop", bufs=1) as wop,
                tc.tile_pool(name="pp", bufs=8) as ppp,
                tc.tile_pool(name="rcp", bufs=3) as rcp,
                tc.tile_pool(name="osb", bufs=4) as osbp,
                tc.tile_pool(name="mblk", bufs=4) as mblkp,
                tc.tile_pool(name="sp", bufs=4, space="PSUM") as spp,
                tc.tile_pool(name="acc", bufs=2, space="PSUM") as accp,
            ):
                wo = wop.tile([128, HPC, D_MODEL], BF16, tag="wo")
                for oc in range(D_MODEL // 512):
                    nc.sync.dma_start(
                        out=wo[:, :, oc * 512 : (oc + 1) * 512],
                        in_=woT_v[:, :, oc * 512 : (oc + 1) * 512],
                    )

                evict_flip = [0]

                for qc in range(NQC):
                    q_lo = qc * 512
                    nki_here = (4 * qc + 4) if causal else NKI
                    for h in range(HPC):
                        sav = accp.tile([128, 1024], F32, tag="acc")
                        sums = sav[:, 0:512]
                        avp = sav[:, 512:1024]
                        for ki in range(nki_here):
                            diag = causal and ki >= 4 * qc
                            q0 = 128 * (ki - 4 * qc) if diag else 0
                            spb = spp.tile([128, 512], F32, tag="sp")
                            pp = ppp.tile([128, 512], BF16, tag="pp")
                            nc.tensor.matmul(
                                spb[:, q0:512],
                                k_rot[:, h, ki * 128 : (ki + 1) * 128],
                                q_rot[:, h, q_lo + q0 : q_lo + 512],
                                start=True,
                                stop=True,
                            )
                            nc.scalar.activation(
                                pp[:, q0:512],
                                spb[:, q0:512],
                                mybir.ActivationFunctionType.Exp,
                                scale=float(exp_scale),
                            )
                            if causal:
                                if diag:
                                    nc.vector.tensor_mul(
                                        pp[:, q0 : q0 + 128],
                                        pp[:, q0 : q0 + 128],
                                        maskd[:, ki * 128 : (ki + 1) * 128],
                                    )
                            else:
                                mb = mblkp.tile([128, 512], BF16, tag="mblk")
                                nc.sync.dma_start(
                                    out=mb[:, :],
                                    in_=maskf_v[:, ki, q_lo : q_lo + 512],
                                )
                                nc.vector.tensor_mul(
                                    pp[:, 0:512], pp[:, 0:512], mb[:, :]
                                )
                            nc.tensor.matmul(
                                sums[:, q0:512],
                                ones_t[:, :],
                                pp[:, q0:512],
                                start=(ki == 0),
                                stop=(ki == nki_here - 1),
                            )
                            nc.tensor.matmul(
                                avp[:, q0:512],
                                v_sb[:, ki, h * 128 : (h + 1) * 128],
                                pp[:, q0:512],
                                start=(ki == 0),
                                stop=(ki == nki_here - 1),
                            )
                        rc = rcp.tile([128, 512], F32, tag="rc")
                        nc.vector.reciprocal_approx_fast(rc[:, :], sums[:, :])
                        nc.vector.tensor_mul(
                            aoT[:, h, q_lo : q_lo + 512], avp[:, :], rc[:, :]
                        )

                    # out-projection for this query chunk (4 seq blocks);
                    # h outer over oc pairs so each aoT stationary load
                    # serves two matmuls
                    for sb in range(4 * qc, 4 * qc + 4):
                        for oc2 in range(2):
                            op2 = accp.tile([128, 1024], F32, tag="acc")
                            for h in range(HPC):
                                lhsT = aoT[:, h, sb * 128 : (sb + 1) * 128]
                                nc.tensor.matmul(
                                    op2[:, 0:512],
                                    lhsT,
                                    wo[:, h, (2 * oc2) * 512 : (2 * oc2 + 1) * 512],
                                    start=(h == 0),
                                    stop=(h == HPC - 1),
                                )
                                nc.tensor.matmul(
                                    op2[:, 512:1024],
                                    lhsT,
                                    wo[:, h, (2 * oc2 + 1) * 512 : (2 * oc2 + 2) * 512],
                                    start=(h == 0),
                                    stop=(h == HPC - 1),
                                )
                            ob = osbp.tile([128, 1024], BF16, tag="osb")
                            if evict_flip[0] % 2 == 0:
                                nc.scalar.copy(ob[:, :], op2[:, :])
                            else:
                                nc.vector.tensor_copy(ob[:, :], op2[:, :])
                            evict_flip[0] += 1
                            nc.sync.dma_start(
                                out=out_d[
                                    sb * 128 : (sb + 1) * 128,
                                    oc2 * 1024 : (oc2 + 1) * 1024,
                                ],
                                in_=ob[:, :],
                            )

    nc.finalize()
    return nc


def _bit_quantize_ternary(w: np.ndarray):
    """Returns (ternary {-1,0,1} float32 matrix, scale) matching the reference."""
    scale = np.maximum(np.mean(np.abs(w.astype(np.float32))), np.float32(1e-5))
    t = np.clip(np.round(w.astype(np.float32) / scale), -1.0, 1.0).astype(np.float32)
    return t, float(scale)


def _host_tables():
    """cos/sin stacked [128, S]: rows 0:64 cos, rows 64:128 sin."""
    inv_freq = 1.0 / (ROPE_BASE ** (np.arange(0, D_HEAD, 2, dtype=np.float32) / D_HEAD))
    pos = np.arange(SEQ, dtype=np.float32)
    ang = pos[:, None] * inv_freq[None, :]  # [S, 64]
    cs = np.empty((128, SEQ), dtype=NPBF16)
    cs[0:64] = np.ascontiguousarray(np.cos(ang).T).astype(NPBF16)
    cs[64:128] = np.ascontiguousarray(np.sin(ang).T).astype(NPBF16)
    sc = np.empty((128, SEQ), dtype=NPBF16)
    sc[0:64] = cs[64:128]
    sc[64:128] = cs[0:64]
    return cs, sc


def kernel(x, w_qkv, w_out, mask):
    global LAST_RESULT
    x = np.asarray(x, dtype=np.float32)
    w_qkv = np.asarray(w_qkv, dtype=np.float32)
    w_out = np.asarray(w_out, dtype=np.float32)
    mask = np.asarray(mask)

    tq, sq = _bit_quantize_ternary(w_qkv)
    to, so = _bit_quantize_ternary(w_out)
    exp_scale = (sq * sq) / float(np.sqrt(D_HEAD))
    c2 = np.float32(sq * so)

    m2 = (mask.reshape(SEQ, SEQ) != 0).astype(np.float32)
    causal = bool(np.array_equal(m2, np.tril(np.ones((SEQ, SEQ), np.float32))))

    cs, sc = _host_tables()
    if causal:
        maskd = np.empty((128, SEQ), dtype=NPBF16)
        for ki in range(NKI):
            blk = m2[ki * 128 : (ki + 1) * 128, ki * 128 : (ki + 1) * 128]  # [q, k]
            maskd[:, ki * 128 : (ki + 1) * 128] = np.ascontiguousarray(blk.T).astype(
                NPBF16
            )
    else:
        maskf = np.ascontiguousarray(m2.T).astype(NPBF16)  # [kk, qq]

    key = (causal, float(exp_scale))
    if key not in _PROG_CACHE:
        _PROG_CACHE[key] = _build_program(causal, float(exp_scale))
    nc = _PROG_CACHE[key]

    in_maps = []
    for c in range(N_CORES):
        b, g = divmod(c, 4)
        rows = slice(R_LOCAL * g, R_LOCAL * (g + 1))
        im = {
            "xT": np.ascontiguousarray(x[b].T).astype(NPBF16),
            "wqT": np.ascontiguousarray(tq[0 * D_MODEL :][rows].T).astype(NPFP8),
            "wkT": np.ascontiguousarray(tq[1 * D_MODEL :][rows].T).astype(NPFP8),
            "wvT": np.ascontiguousarray(tq[2 * D_MODEL :][rows].T).astype(NPFP8),
            "woT": np.ascontiguousarray(to[:, rows].T).astype(NPBF16),
            "cossinT": cs,
            "sincosT": sc,
        }
        if causal:
            im["maskd"] = maskd
        else:
            im["maskf"] = maskf
        in_maps.append(im)

    do_trace = bool(PROFILE) and _enable_profiling()
    res = run_bass_kernel_spmd(nc, in_maps, list(range(N_CORES)), trace=do_trace)
    LAST_RESULT = res

    parts = [np.asarray(res.results[c]["out"]).astype(np.float32) for c in range(N_CORES)]
    out = np.stack(
        [
            parts[0] + parts[1] + parts[2] + parts[3],
            parts[4] + parts[5] + parts[6] + parts[7],
        ]
    )
    return (out * c2).astype(np.float32)



# revision 2
# speedup vs baseline: 1.1522x; 1.1522x over previous
"""Trainium2 Bass kernel for BitNet multi-head attention (nn_MultiHeadAttention_62294205661880).

Sharding: 8 cores = 2 batches x 4 head-groups (4 heads each).  Each core
computes qkv projection, RoPE, causal attention and a column-parallel slice
of the output projection for its (batch, head-group); the host sums the 4
partial out-projections per batch (the tensor-parallel all-reduce done
host-side, since the contract gathers to host anyway).

BitNet quantization is folded on the host: weights are uploaded as exact
ternary {-1,0,+1} matrices; scale_qkv^2/sqrt(dh) is folded into the
softmax exp() scale and scale_qkv*scale_out into a final host-side scalar.

Precision/speed split:
 - Q/K projections run as fp8e4m3 DoubleRow matmuls (2 contraction blocks
   per pass): x and the exact-ternary weights are fp8.  The fp8 noise on
   q/k is softened by softmax normalization (measured ~1e-2 rel err).
 - Everything else (V path, scores, attention, out-projection) runs in
   fp16 (same PE speed as bf16, 8x the mantissa) to keep margin.
 - The softmax denominator is accumulated across key blocks on the Vector
   engine (fp16 adds) and reduced over partitions with a single ones-
   matmul per (query chunk, head) instead of one matmul per key block.

Device layout trick: everything is computed transposed.  Q_T/K_T come out of
the projection as [dh, S]; scores are computed as s_T[k, q]; AV produces
out_T[dh, q] which feeds the output projection directly.  No on-device
transposes at all.  Softmax skips the max-subtraction: scores are bounded
(~+-2) because the BitNet weight scale is tiny, so exp() is safe.
"""

import sys
import types

import numpy as np
import ml_dtypes

import concourse.bass as bass
import concourse.mybir as mybir
import concourse.tile as tile
from concourse import bacc
from concourse.bass_utils import run_bass_kernel_spmd

D_MODEL = 2048
N_HEADS = 16
D_HEAD = 128
SEQ = 2048
BATCH = 2
ROPE_BASE = 10000.0

N_CORES = 8
HPC = 4  # heads per core
R_LOCAL = HPC * D_HEAD  # 512 local q (or k, or v) rows per core
MO = D_MODEL // 128  # 16 contraction blocks
MP = MO // 2  # 8 contraction block pairs (DoubleRow)
NKI = SEQ // 128  # 16 key blocks
NQC = SEQ // 512  # 4 query chunks of 512
NSB = SEQ // 128  # 16 seq blocks (v / proj)

F16 = mybir.dt.float16
F32 = mybir.dt.float32
NPF16 = np.float16
NPFP8 = ml_dtypes.float8_e4m3
FP8 = mybir.dt.float8e4
DR = mybir.MatmulPerfMode.DoubleRow

LAST_RESULT = None  # BassKernelResults of the most recent run (for test.py)
_PROG_CACHE = {}
PROFILE = False  # test.py sets True to capture an NTFF profile / HW exec time


def _enable_profiling() -> bool:
    """Install the axon NTFF profile hook glue if the image lacks
    ``antenv.axon_hooks`` (boot degrades silently without it), and skip
    the artifact upload (no bucket access in this container)."""
    try:
        from antenv.axon_hooks import get_axon_ntff_profile_hook  # noqa: F401

        ok = get_axon_ntff_profile_hook() is not None
    except ImportError:
        ok = False
        import antenv

        mod = types.ModuleType("antenv.axon_hooks")
        mod._hook = None
        mod.set_axon_ntff_profile_hook = lambda h: setattr(mod, "_hook", h)
        mod.get_axon_ntff_profile_hook = lambda: mod._hook
        sys.modules["antenv.axon_hooks"] = mod
        antenv.axon_hooks = mod
        try:
            from trn_agent_boot.trn_boot import _ntff_profile_via_ctypes

            hook = _ntff_profile_via_ctypes("/opt/axon/libaxon_pjrt.so")
            if hook is not None:
                mod._hook = hook
                ok = True
        except Exception as e:  # profiling is best-effort
            print(f"ntff profile hook install failed: {e}", file=sys.stderr)
    if ok:
        import concourse.bass_utils as _bu

        _bu.upload_artifacts = lambda tmpdir: tmpdir
    return ok


def _build_program(causal: bool, exp_scale: float) -> bass.Bass:
    nc = bacc.Bacc(None)
    S = SEQ

    xT8_d = nc.dram_tensor("xT8", [D_MODEL, S], FP8, kind="ExternalInput")
    xT16_d = nc.dram_tensor("xT16", [D_MODEL, S], F16, kind="ExternalInput")
    wqT_d = nc.dram_tensor("wqT", [D_MODEL, R_LOCAL], FP8, kind="ExternalInput")
    wkT_d = nc.dram_tensor("wkT", [D_MODEL, R_LOCAL], FP8, kind="ExternalInput")
    wvT_d = nc.dram_tensor("wvT", [D_MODEL, R_LOCAL], F16, kind="ExternalInput")
    woT_d = nc.dram_tensor("woT", [R_LOCAL, D_MODEL], F16, kind="ExternalInput")
    # cos rows 0:64, sin rows 64:128
    cs_d = nc.dram_tensor("cossinT", [128, S], F16, kind="ExternalInput")
    # swapped: sin rows 0:64, cos rows 64:128 (keeps TensorTensor base partitions equal)
    sc_d = nc.dram_tensor("sincosT", [128, S], F16, kind="ExternalInput")
    if causal:
        # 16 transposed diagonal 128x128 mask blocks, side by side
        maskd_d = nc.dram_tensor("maskd", [128, S], F16, kind="ExternalInput")
    else:
        maskf_d = nc.dram_tensor("maskf", [S, S], F16, kind="ExternalInput")
    out_d = nc.dram_tensor("out", [S, D_MODEL], F16, kind="ExternalOutput")

    xT8_v = xT8_d[:].rearrange("(mo p) s -> p mo s", p=128)
    xT16_v = xT16_d[:].rearrange("(mo p) s -> p mo s", p=128)
    wqT_v = wqT_d[:].rearrange("(mo p) r -> p mo r", p=128)
    wkT_v = wkT_d[:].rearrange("(mo p) r -> p mo r", p=128)
    wvT_v = wvT_d[:].rearrange("(mo p) r -> p mo r", p=128)
    woT_v = woT_d[:].rearrange("(h p) o -> p h o", p=128)
    if not causal:
        maskf_v = maskf_d[:].rearrange("(ko p) q -> p ko q", p=128)

    with tile.TileContext(nc) as tc:
        with tc.tile_pool(name="pers", bufs=1) as pers:
            # ---- persistent SBUF tensors (live across both phases) ----
            q_rot = pers.tile([128, HPC, S], F16, tag="qrot")
            k_rot = pers.tile([128, HPC, S], F16, tag="krot")
            v_sb = pers.tile([128, NKI, R_LOCAL], F16, tag="vsb")
            ones_t = pers.tile([128, 128], F16, tag="ones")
            warm = pers.tile([128, 1], F16, tag="warm")
            if causal:
                maskd = pers.tile([128, S], F16, tag="maskd")
            nc.vector.memset(ones_t[:, :], 1.0)
            # load the exp table set first so no ACT table switch happens
            # mid-kernel (Copy lives in every set).
            nc.scalar.activation(
                warm[:, :], ones_t[:, 0:1], mybir.ActivationFunctionType.Exp
            )

            # ================= phase A: QKV projection + RoPE =================
            with (
                tc.tile_pool(name="xtp", bufs=1) as xtp,
                tc.tile_pool(name="xvq", bufs=2) as xvqp,
                tc.tile_pool(name="wp", bufs=1) as wp,
                tc.tile_pool(name="raw", bufs=2) as rawp,
                tc.tile_pool(name="tmp", bufs=2) as tmpp,
                tc.tile_pool(name="psA", bufs=2, space="PSUM") as psA,
            ):
                xt8 = xtp.tile([128, MO, S], FP8, tag="xt8")
                wq = wp.tile([128, MO, R_LOCAL], FP8, tag="wq")
                wk = wp.tile([128, MO, R_LOCAL], FP8, tag="wk")
                wv = wp.tile([128, MO, R_LOCAL], F16, tag="wv")
                cs_t = wp.tile([128, S], F16, tag="cs")
                sc_t = wp.tile([128, S], F16, tag="sc")

                for mo in range(MO):
                    nc.sync.dma_start(out=wq[:, mo, :], in_=wqT_v[:, mo, :])
                    nc.sync.dma_start(out=wk[:, mo, :], in_=wkT_v[:, mo, :])
                    if mo < 4:
                        nc.sync.dma_start(
                            out=xt8[:, mo, 0:1024], in_=xT8_v[:, mo, 0:1024]
                        )
                        nc.sync.dma_start(
                            out=xt8[:, mo, 1024:2048], in_=xT8_v[:, mo, 1024:2048]
                        )
                    else:
                        nc.sync.dma_start(out=xt8[:, mo, :], in_=xT8_v[:, mo, :])
                nc.sync.dma_start(out=cs_t[:, :], in_=cs_d[:, :])
                nc.sync.dma_start(out=sc_t[:, :], in_=sc_d[:, :])
                if causal:
                    nc.sync.dma_start(out=maskd[:, :], in_=maskd_d[:, :])
                for mo in range(MO):
                    nc.sync.dma_start(out=wv[:, mo, :], in_=wvT_v[:, mo, :])

                def rope(dst, raw):
                    """NeoX rotary: rows 0:64 = t*c - b*s ; rows 64:128 = t*s + b*c."""
                    ta = tmpp.tile([64, S], F16, tag="tmp")
                    tb = tmpp.tile([64, S], F16, tag="tmp")
                    nc.vector.tensor_mul(ta[:, :], raw[0:64, :], cs_t[0:64, :])
                    nc.vector.tensor_mul(tb[:, :], raw[64:128, :], cs_t[64:128, :])
                    nc.vector.tensor_sub(dst[0:64, :], ta[:, :], tb[:, :])
                    tc2 = tmpp.tile([64, S], F16, tag="tmp")
                    td = tmpp.tile([64, S], F16, tag="tmp")
                    nc.vector.tensor_mul(tc2[:, :], raw[0:64, :], sc_t[0:64, :])
                    nc.vector.tensor_mul(td[:, :], raw[64:128, :], sc_t[64:128, :])
                    nc.vector.tensor_add(dst[64:128, :], tc2[:, :], td[:, :])

                # head 0 q/k with the m-pair loop OUTER so the matmuls consume
                # xt8 m-blocks as the DMAs land (startup overlap).
                qp0 = psA.tile([128, S], F32, tag="psA")
                kp0 = psA.tile([128, S], F32, tag="psA")
                for mp in range(MP):
                    for c4 in range(4):
                        nc.tensor.matmul(
                            qp0[:, c4 * 512 : (c4 + 1) * 512],
                            wq[:, 2 * mp : 2 * mp + 2, 0:128],
                            xt8[:, 2 * mp : 2 * mp + 2, c4 * 512 : (c4 + 1) * 512],
                            start=(mp == 0),
                            stop=(mp == MP - 1),
                            perf_mode=DR,
                        )
                        nc.tensor.matmul(
                            kp0[:, c4 * 512 : (c4 + 1) * 512],
                            wk[:, 2 * mp : 2 * mp + 2, 0:128],
                            xt8[:, 2 * mp : 2 * mp + 2, c4 * 512 : (c4 + 1) * 512],
                            start=(mp == 0),
                            stop=(mp == MP - 1),
                            perf_mode=DR,
                        )
                q_raw = rawp.tile([128, S], F16, tag="raw")
                nc.scalar.copy(q_raw[:, :], qp0[:, :])
                rope(q_rot[:, 0, :], q_raw)
                k_raw = rawp.tile([128, S], F16, tag="raw")
                nc.scalar.copy(k_raw[:, :], kp0[:, :])
                rope(k_rot[:, 0, :], k_raw)

                def project(dst_raw, w_sb, h):
                    """q/k head projection -> fp16 raw [128, S] (xt8 resident)."""
                    ps = psA.tile([128, S], F32, tag="psA")
                    for c4 in range(4):
                        for mp in range(MP):
                            nc.tensor.matmul(
                                ps[:, c4 * 512 : (c4 + 1) * 512],
                                w_sb[:, 2 * mp : 2 * mp + 2, h * 128 : (h + 1) * 128],
                                xt8[:, 2 * mp : 2 * mp + 2, c4 * 512 : (c4 + 1) * 512],
                                start=(mp == 0),
                                stop=(mp == MP - 1),
                                perf_mode=DR,
                            )
                    nc.scalar.copy(dst_raw[:, :], ps[:, :])

                for h in range(1, HPC):
                    q_raw = rawp.tile([128, S], F16, tag="raw")
                    project(q_raw, wq, h)
                    rope(q_rot[:, h, :], q_raw)
                    k_raw = rawp.tile([128, S], F16, tag="raw")
                    project(k_raw, wk, h)
                    rope(k_rot[:, h, :], k_raw)

                # V projection (natural layout [s, r]) in fp16; x quarters are
                # streamed through a double-buffered pool (SBUF can't hold the
                # fp16 x alongside the fp8 copy).
                for sb4 in range(NSB // 4):
                    xq = xvqp.tile([128, MO, 512], F16, tag="xq")
                    nc.sync.dma_start(
                        out=xq[:, :, :], in_=xT16_v[:, :, sb4 * 512 : (sb4 + 1) * 512]
                    )
                    ps = psA.tile([128, S], F32, tag="psA")
                    for part in range(4):
                        for m in range(MO):
                            nc.tensor.matmul(
                                ps[:, part * 512 : part * 512 + 512],
                                xq[:, m, part * 128 : (part + 1) * 128],
                                wv[:, m, :],
                                start=(m == 0),
                                stop=(m == MO - 1),
                            )
                    nc.scalar.copy(v_sb[:, sb4 * 4 : sb4 * 4 + 4, :], ps[:, :])

            # ================= phase B: attention + out-projection =============
            with (
                tc.tile_pool(name="wop", bufs=1) as wop,
                tc.tile_pool(name="pp", bufs=8) as ppp,
                tc.tile_pool(name="pac", bufs=2) as pacp,
                tc.tile_pool(name="ao", bufs=2) as aop,
                tc.tile_pool(name="rcp", bufs=3) as rcp,
                tc.tile_pool(name="osb", bufs=4) as osbp,
                tc.tile_pool(name="mblk", bufs=4) as mblkp,
                tc.tile_pool(name="sp", bufs=4, space="PSUM") as spp,
                tc.tile_pool(name="acc", bufs=2, space="PSUM") as accp,
            ):
                wo = wop.tile([128, HPC, D_MODEL], F16, tag="wo")
                for oc in range(D_MODEL // 512):
                    nc.sync.dma_start(
                        out=wo[:, :, oc * 512 : (oc + 1) * 512],
                        in_=woT_v[:, :, oc * 512 : (oc + 1) * 512],
                    )

                evict_flip = [0]

                for qc in range(NQC):
                    q_lo = qc * 512
                    nki_here = (4 * qc + 4) if causal else NKI
                    aoT = aop.tile([128, HPC, 512], F16, tag="aoT")
                    for h in range(HPC):
                        sav = accp.tile([128, 1024], F32, tag="acc")
                        sums = sav[:, 0:512]
                        avp = sav[:, 512:1024]
                        pacc = pacp.tile([128, 512], F16, tag="pac")
                        for ki in range(nki_here):
                            diag = causal and ki >= 4 * qc
                            q0 = 128 * (ki - 4 * qc) if diag else 0
                            spb = spp.tile([128, 512], F32, tag="sp")
                            pp = ppp.tile([128, 512], F16, tag="pp")
                            nc.tensor.matmul(
                                spb[:, q0:512],
                                k_rot[:, h, ki * 128 : (ki + 1) * 128],
                                q_rot[:, h, q_lo + q0 : q_lo + 512],
                                start=True,
                                stop=True,
                            )
                            nc.scalar.activation(
                                pp[:, q0:512],
                                spb[:, q0:512],
                                mybir.ActivationFunctionType.Exp,
                                scale=float(exp_scale),
                            )
                            if causal:
                                if diag:
                                    nc.vector.tensor_mul(
                                        pp[:, q0 : q0 + 128],
                                        pp[:, q0 : q0 + 128],
                                        maskd[:, ki * 128 : (ki + 1) * 128],
                                    )
                            else:
                                mb = mblkp.tile([128, 512], F16, tag="mblk")
                                nc.sync.dma_start(
                                    out=mb[:, :],
                                    in_=maskf_v[:, ki, q_lo : q_lo + 512],
                                )
                                nc.vector.tensor_mul(
                                    pp[:, 0:512], pp[:, 0:512], mb[:, :]
                                )
                            # fold this key block into the softmax denominator
                            # accumulator (vector engine; partition reduce via a
                            # single ones-matmul at the end of the ki loop).
                            if ki == 0:
                                nc.vector.tensor_copy(pacc[:, :], pp[:, :])
                            else:
                                nc.vector.tensor_add(
                                    pacc[:, q0:512], pacc[:, q0:512], pp[:, q0:512]
                                )
                            nc.tensor.matmul(
                                avp[:, q0:512],
                                v_sb[:, ki, h * 128 : (h + 1) * 128],
                                pp[:, q0:512],
                                start=(ki == 0),
                                stop=(ki == nki_here - 1),
                            )
                        nc.tensor.matmul(
                            sums[:, :],
                            ones_t[:, :],
                            pacc[:, :],
                            start=True,
                            stop=True,
                        )
                        rc = rcp.tile([128, 512], F32, tag="rc")
                        nc.vector.reciprocal_approx_fast(rc[:, :], sums[:, :])
                        nc.vector.tensor_mul(
                            aoT[:, h, :], avp[:, :], rc[:, :]
                        )

                    # out-projection for this query chunk (4 seq blocks);
                    # h outer over oc pairs so each aoT stationary load
                    # serves two matmuls
                    for sb_l in range(4):
                        sb = 4 * qc + sb_l
                        for oc2 in range(2):
                            op2 = accp.tile([128, 1024], F32, tag="acc")
                            for h in range(HPC):
                                lhsT = aoT[:, h, sb_l * 128 : (sb_l + 1) * 128]
                                nc.tensor.matmul(
                                    op2[:, 0:512],
                                    lhsT,
                                    wo[:, h, (2 * oc2) * 512 : (2 * oc2 + 1) * 512],
                                    start=(h == 0),
                                    stop=(h == HPC - 1),
                                )
                                nc.tensor.matmul(
                                    op2[:, 512:1024],
                                    lhsT,
                                    wo[:, h, (2 * oc2 + 1) * 512 : (2 * oc2 + 2) * 512],
                                    start=(h == 0),
                                    stop=(h == HPC - 1),
                                )
                            ob = osbp.tile([128, 1024], F16, tag="osb")
                            if evict_flip[0] % 2 == 0:
                                nc.scalar.copy(ob[:, :], op2[:, :])
                            else:
                                nc.vector.tensor_copy(ob[:, :], op2[:, :])
                            evict_flip[0] += 1
                            nc.sync.dma_start(
                                out=out_d[
                                    sb * 128 : (sb + 1) * 128,
                                    oc2 * 1024 : (oc2 + 1) * 1024,
                                ],
                                in_=ob[:, :],
                            )

    nc.finalize()
    return nc


def _bit_quantize_ternary(w: np.ndarray):
    """Returns (ternary {-1,0,1} float32 matrix, scale) matching the reference."""
    scale = np.maximum(np.mean(np.abs(w.astype(np.float32))), np.float32(1e-5))
    t = np.clip(np.round(w.astype(np.float32) / scale), -1.0, 1.0).astype(np.float32)
    return t, float(scale)


def _host_tables():
    """cos/sin stacked [128, S]: rows 0:64 cos, rows 64:128 sin."""
    inv_freq = 1.0 / (ROPE_BASE ** (np.arange(0, D_HEAD, 2, dtype=np.float32) / D_HEAD))
    pos = np.arange(SEQ, dtype=np.float32)
    ang = pos[:, None] * inv_freq[None, :]  # [S, 64]
    cs = np.empty((128, SEQ), dtype=NPF16)
    cs[0:64] = np.ascontiguousarray(np.cos(ang).T).astype(NPF16)
    cs[64:128] = np.ascontiguousarray(np.sin(ang).T).astype(NPF16)
    sc = np.empty((128, SEQ), dtype=NPF16)
    sc[0:64] = cs[64:128]
    sc[64:128] = cs[0:64]
    return cs, sc


def kernel(x, w_qkv, w_out, mask):
    global LAST_RESULT
    x = np.asarray(x, dtype=np.float32)
    w_qkv = np.asarray(w_qkv, dtype=np.float32)
    w_out = np.asarray(w_out, dtype=np.float32)
    mask = np.asarray(mask)

    tq, sq = _bit_quantize_ternary(w_qkv)
    to, so = _bit_quantize_ternary(w_out)
    exp_scale = (sq * sq) / float(np.sqrt(D_HEAD))
    c2 = np.float32(sq * so)

    m2 = (mask.reshape(SEQ, SEQ) != 0).astype(np.float32)
    causal = bool(np.array_equal(m2, np.tril(np.ones((SEQ, SEQ), np.float32))))

    cs, sc = _host_tables()
    if causal:
        maskd = np.empty((128, SEQ), dtype=NPF16)
        for ki in range(NKI):
            blk = m2[ki * 128 : (ki + 1) * 128, ki * 128 : (ki + 1) * 128]  # [q, k]
            maskd[:, ki * 128 : (ki + 1) * 128] = np.ascontiguousarray(blk.T).astype(
                NPF16
            )
    else:
        maskf = np.ascontiguousarray(m2.T).astype(NPF16)  # [kk, qq]

    key = (causal, float(exp_scale))
    if key not in _PROG_CACHE:
        _PROG_CACHE[key] = _build_program(causal, float(exp_scale))
    nc = _PROG_CACHE[key]

    xT8 = [np.ascontiguousarray(x[b].T).astype(NPFP8) for b in range(BATCH)]
    xT16 = [np.ascontiguousarray(x[b].T).astype(NPF16) for b in range(BATCH)]

    in_maps = []
    for c in range(N_CORES):
        b, g = divmod(c, 4)
        rows = slice(R_LOCAL * g, R_LOCAL * (g + 1))
        im = {
            "xT8": xT8[b],
            "xT16": xT16[b],
            "wqT": np.ascontiguousarray(tq[0 * D_MODEL :][rows].T).astype(NPFP8),
            "wkT": np.ascontiguousarray(tq[1 * D_MODEL :][rows].T).astype(NPFP8),
            "wvT": np.ascontiguousarray(tq[2 * D_MODEL :][rows].T).astype(NPF16),
            "woT": np.ascontiguousarray(to[:, rows].T).astype(NPF16),
            "cossinT": cs,
            "sincosT": sc,
        }
        if causal:
            im["maskd"] = maskd
        else:
            im["maskf"] = maskf
        in_maps.append(im)

    do_trace = bool(PROFILE) and _enable_profiling()
    res = run_bass_kernel_spmd(nc, in_maps, list(range(N_CORES)), trace=do_trace)
    LAST_RESULT = res

    parts = [np.asarray(res.results[c]["out"]).astype(np.float32) for c in range(N_CORES)]
    out = np.stack(
        [
            parts[0] + parts[1] + parts[2] + parts[3],
            parts[4] + parts[5] + parts[6] + parts[7],
        ]
    )
    return (out * c2).astype(np.float32)


# revision 4
# speedup vs baseline: 1.1616x; 1.0082x over previous
"""Trainium2 Bass kernel for BitNet multi-head attention (nn_MultiHeadAttention_62294205661880).

Sharding: 8 cores = 2 batches x 4 head-groups (4 heads each).  Each core
computes qkv projection, RoPE, causal attention and a column-parallel slice
of the output projection for its (batch, head-group); the host sums the 4
partial out-projections per batch (the tensor-parallel all-reduce done
host-side, since the contract gathers to host anyway).

BitNet quantization is folded on the host: weights are uploaded as exact
ternary {-1,0,+1} matrices; scale_qkv^2/sqrt(dh) is folded into the
softmax exp() scale and scale_qkv*scale_out into a final host-side scalar.

Precision/speed split:
 - Q/K projections run as fp8e4m3 DoubleRow matmuls (2 contraction blocks
   per pass): x and the exact-ternary weights are fp8.  The fp8 noise on
   q/k is softened by softmax normalization (measured ~1e-2 rel err).
 - Everything else (V path, scores, attention, out-projection) runs in
   fp16 (same PE speed as bf16, 8x the mantissa) to keep margin.
 - The softmax denominator is accumulated across key blocks on the Vector
   engine (fp16 adds) and reduced over partitions with a single ones-
   matmul per (query chunk, head) instead of one matmul per key block.
   That ones-matmul is emitted lazily (inside the NEXT head's block loop)
   so the PE never waits on the Vector engine's accumulator chain.
 - Full (non-diagonal) key blocks are processed in pairs so each Exp
   activation covers 1024 columns (the ACT engine would otherwise pace
   the attention inner loop).

Device layout trick: everything is computed transposed.  Q_T/K_T come out of
the projection as [dh, S]; scores are computed as s_T[k, q]; AV produces
out_T[dh, q] which feeds the output projection directly.  No on-device
transposes at all.  Softmax skips the max-subtraction: scores are bounded
(~+-2) because the BitNet weight scale is tiny, so exp() is safe.
"""

import sys
import types

import numpy as np
import ml_dtypes

import concourse.bass as bass
import concourse.mybir as mybir
import concourse.tile as tile
from concourse import bacc
from concourse.bass_utils import run_bass_kernel_spmd

D_MODEL = 2048
N_HEADS = 16
D_HEAD = 128
SEQ = 2048
BATCH = 2
ROPE_BASE = 10000.0

N_CORES = 8
HPC = 4  # heads per core
R_LOCAL = HPC * D_HEAD  # 512 local q (or k, or v) rows per core
MO = D_MODEL // 128  # 16 contraction blocks
MP = MO // 2  # 8 contraction block pairs (DoubleRow)
NKI = SEQ // 128  # 16 key blocks
NQC = SEQ // 512  # 4 query chunks of 512
NSB = SEQ // 128  # 16 seq blocks (v / proj)
N_WARM = 80  # PE warm-up matmuls issued while the first DMAs land

F16 = mybir.dt.float16
F32 = mybir.dt.float32
NPF16 = np.float16
NPFP8 = ml_dtypes.float8_e4m3
FP8 = mybir.dt.float8e4
DR = mybir.MatmulPerfMode.DoubleRow

LAST_RESULT = None  # BassKernelResults of the most recent run (for test.py)
_PROG_CACHE = {}
PROFILE = False  # test.py sets True to capture an NTFF profile / HW exec time


def _enable_profiling() -> bool:
    """Install the axon NTFF profile hook glue if the image lacks
    ``antenv.axon_hooks`` (boot degrades silently without it), and skip
    the artifact upload (no bucket access in this container)."""
    try:
        from antenv.axon_hooks import get_axon_ntff_profile_hook  # noqa: F401

        ok = get_axon_ntff_profile_hook() is not None
    except ImportError:
        ok = False
        import antenv

        mod = types.ModuleType("antenv.axon_hooks")
        mod._hook = None
        mod.set_axon_ntff_profile_hook = lambda h: setattr(mod, "_hook", h)
        mod.get_axon_ntff_profile_hook = lambda: mod._hook
        sys.modules["antenv.axon_hooks"] = mod
        antenv.axon_hooks = mod
        try:
            from trn_agent_boot.trn_boot import _ntff_profile_via_ctypes

            hook = _ntff_profile_via_ctypes("/opt/axon/libaxon_pjrt.so")
            if hook is not None:
                mod._hook = hook
                ok = True
        except Exception as e:  # profiling is best-effort
            print(f"ntff profile hook install failed: {e}", file=sys.stderr)
    if ok:
        import concourse.bass_utils as _bu

        _bu.upload_artifacts = lambda tmpdir: tmpdir
    return ok


def _build_program(causal: bool, exp_scale: float) -> bass.Bass:
    nc = bacc.Bacc(None)
    S = SEQ

    xT8_d = nc.dram_tensor("xT8", [D_MODEL, S], FP8, kind="ExternalInput")
    xT16_d = nc.dram_tensor("xT16", [D_MODEL, S], F16, kind="ExternalInput")
    wqT_d = nc.dram_tensor("wqT", [D_MODEL, R_LOCAL], FP8, kind="ExternalInput")
    wkT_d = nc.dram_tensor("wkT", [D_MODEL, R_LOCAL], FP8, kind="ExternalInput")
    wvT_d = nc.dram_tensor("wvT", [D_MODEL, R_LOCAL], F16, kind="ExternalInput")
    woT_d = nc.dram_tensor("woT", [R_LOCAL, D_MODEL], F16, kind="ExternalInput")
    # cos rows 0:64, sin rows 64:128
    cs_d = nc.dram_tensor("cossinT", [128, S], F16, kind="ExternalInput")
    # swapped: sin rows 0:64, cos rows 64:128 (keeps TensorTensor base partitions equal)
    sc_d = nc.dram_tensor("sincosT", [128, S], F16, kind="ExternalInput")
    if causal:
        # 16 transposed diagonal 128x128 mask blocks, side by side
        maskd_d = nc.dram_tensor("maskd", [128, S], F16, kind="ExternalInput")
    else:
        maskf_d = nc.dram_tensor("maskf", [S, S], F16, kind="ExternalInput")
    out_d = nc.dram_tensor("out", [S, D_MODEL], F16, kind="ExternalOutput")

    xT8_v = xT8_d[:].rearrange("(mo p) s -> p mo s", p=128)
    xT16_v = xT16_d[:].rearrange("(mo p) s -> p mo s", p=128)
    wqT_v = wqT_d[:].rearrange("(mo p) r -> p mo r", p=128)
    wkT_v = wkT_d[:].rearrange("(mo p) r -> p mo r", p=128)
    wvT_v = wvT_d[:].rearrange("(mo p) r -> p mo r", p=128)
    woT_v = woT_d[:].rearrange("(h p) o -> p h o", p=128)
    if not causal:
        maskf_v = maskf_d[:].rearrange("(ko p) q -> p ko q", p=128)

    with tile.TileContext(nc) as tc:
        with tc.tile_pool(name="pers", bufs=1) as pers:
            # ---- persistent SBUF tensors (live across both phases) ----
            q_rot = pers.tile([128, HPC, S], F16, tag="qrot")
            k_rot = pers.tile([128, HPC, S], F16, tag="krot")
            v_sb = pers.tile([128, NKI, R_LOCAL], F16, tag="vsb")
            ones_t = pers.tile([128, 128], F16, tag="ones")
            warm = pers.tile([128, 1], F16, tag="warm")
            if causal:
                maskd = pers.tile([128, S], F16, tag="maskd")
            nc.vector.memset(ones_t[:, :], 1.0)
            # load the exp table set first so no ACT table switch happens
            # mid-kernel (Copy lives in every set).
            nc.scalar.activation(
                warm[:, :], ones_t[:, 0:1], mybir.ActivationFunctionType.Exp
            )

            # ================= phase A: QKV projection + RoPE =================
            with (
                tc.tile_pool(name="xtp", bufs=1) as xtp,
                tc.tile_pool(name="xvq", bufs=2) as xvqp,
                tc.tile_pool(name="wp", bufs=1) as wp,
                tc.tile_pool(name="raw", bufs=2) as rawp,
                tc.tile_pool(name="tmp", bufs=2) as tmpp,
                tc.tile_pool(name="psA", bufs=2, space="PSUM") as psA,
            ):
                xt8 = xtp.tile([128, MO, S], FP8, tag="xt8")
                wq = wp.tile([128, MO, R_LOCAL], FP8, tag="wq")
                wk = wp.tile([128, MO, R_LOCAL], FP8, tag="wk")
                wv = wp.tile([128, MO, R_LOCAL], F16, tag="wv")
                cs_t = wp.tile([128, S], F16, tag="cs")
                sc_t = wp.tile([128, S], F16, tag="sc")

                # warm up the PE / HAM while the first input DMAs land
                warmps = psA.tile([128, 128], F32, tag="psA")
                for _ in range(N_WARM):
                    nc.tensor.matmul(
                        warmps[:, :],
                        ones_t[:, :],
                        ones_t[:, :],
                        start=True,
                        stop=True,
                    )

                # DMA order: the first two m-pairs' weights/x first (they gate
                # the first real matmul), then the rest coarsely batched.
                for mo in range(4):
                    nc.sync.dma_start(out=wq[:, mo, :], in_=wqT_v[:, mo, :])
                    nc.sync.dma_start(out=wk[:, mo, :], in_=wkT_v[:, mo, :])
                    nc.sync.dma_start(out=xt8[:, mo, 0:1024], in_=xT8_v[:, mo, 0:1024])
                    nc.sync.dma_start(
                        out=xt8[:, mo, 1024:2048], in_=xT8_v[:, mo, 1024:2048]
                    )
                for g4 in range(1, 4):
                    sl = slice(4 * g4, 4 * g4 + 4)
                    nc.sync.dma_start(out=wq[:, sl, :], in_=wqT_v[:, sl, :])
                    nc.sync.dma_start(out=wk[:, sl, :], in_=wkT_v[:, sl, :])
                    nc.sync.dma_start(out=xt8[:, sl, :], in_=xT8_v[:, sl, :])
                nc.sync.dma_start(out=cs_t[:, :], in_=cs_d[:, :])
                nc.sync.dma_start(out=sc_t[:, :], in_=sc_d[:, :])
                if causal:
                    nc.sync.dma_start(out=maskd[:, :], in_=maskd_d[:, :])
                for g8 in range(2):
                    sl = slice(8 * g8, 8 * g8 + 8)
                    nc.sync.dma_start(out=wv[:, sl, :], in_=wvT_v[:, sl, :])

                def rope(dst, raw):
                    """NeoX rotary: rows 0:64 = t*c - b*s ; rows 64:128 = t*s + b*c."""
                    ta = tmpp.tile([64, S], F16, tag="tmp")
                    tb = tmpp.tile([64, S], F16, tag="tmp")
                    nc.vector.tensor_mul(ta[:, :], raw[0:64, :], cs_t[0:64, :])
                    nc.vector.tensor_mul(tb[:, :], raw[64:128, :], cs_t[64:128, :])
                    nc.vector.tensor_sub(dst[0:64, :], ta[:, :], tb[:, :])
                    tc2 = tmpp.tile([64, S], F16, tag="tmp")
                    td = tmpp.tile([64, S], F16, tag="tmp")
                    nc.vector.tensor_mul(tc2[:, :], raw[0:64, :], sc_t[0:64, :])
                    nc.vector.tensor_mul(td[:, :], raw[64:128, :], sc_t[64:128, :])
                    nc.vector.tensor_add(dst[64:128, :], tc2[:, :], td[:, :])

                # head 0 q/k with the m-pair loop OUTER so the matmuls consume
                # xt8 m-blocks as the DMAs land (startup overlap).
                qp0 = psA.tile([128, S], F32, tag="psA")
                kp0 = psA.tile([128, S], F32, tag="psA")
                for mp in range(MP):
                    for c4 in range(4):
                        nc.tensor.matmul(
                            qp0[:, c4 * 512 : (c4 + 1) * 512],
                            wq[:, 2 * mp : 2 * mp + 2, 0:128],
                            xt8[:, 2 * mp : 2 * mp + 2, c4 * 512 : (c4 + 1) * 512],
                            start=(mp == 0),
                            stop=(mp == MP - 1),
                            perf_mode=DR,
                        )
                        nc.tensor.matmul(
                            kp0[:, c4 * 512 : (c4 + 1) * 512],
                            wk[:, 2 * mp : 2 * mp + 2, 0:128],
                            xt8[:, 2 * mp : 2 * mp + 2, c4 * 512 : (c4 + 1) * 512],
                            start=(mp == 0),
                            stop=(mp == MP - 1),
                            perf_mode=DR,
                        )
                q_raw = rawp.tile([128, S], F16, tag="raw")
                nc.scalar.copy(q_raw[:, :], qp0[:, :])
                rope(q_rot[:, 0, :], q_raw)
                k_raw = rawp.tile([128, S], F16, tag="raw")
                nc.scalar.copy(k_raw[:, :], kp0[:, :])
                rope(k_rot[:, 0, :], k_raw)

                def project(dst_raw, w_sb, h):
                    """q/k head projection -> fp16 raw [128, S] (xt8 resident)."""
                    ps = psA.tile([128, S], F32, tag="psA")
                    for c4 in range(4):
                        for mp in range(MP):
                            nc.tensor.matmul(
                                ps[:, c4 * 512 : (c4 + 1) * 512],
                                w_sb[:, 2 * mp : 2 * mp + 2, h * 128 : (h + 1) * 128],
                                xt8[:, 2 * mp : 2 * mp + 2, c4 * 512 : (c4 + 1) * 512],
                                start=(mp == 0),
                                stop=(mp == MP - 1),
                                perf_mode=DR,
                            )
                    nc.scalar.copy(dst_raw[:, :], ps[:, :])

                for h in range(1, HPC):
                    q_raw = rawp.tile([128, S], F16, tag="raw")
                    project(q_raw, wq, h)
                    rope(q_rot[:, h, :], q_raw)
                    k_raw = rawp.tile([128, S], F16, tag="raw")
                    project(k_raw, wk, h)
                    rope(k_rot[:, h, :], k_raw)

                # V projection (natural layout [s, r]) in fp16; x quarters are
                # streamed through a double-buffered pool (SBUF can't hold the
                # fp16 x alongside the fp8 copy).
                for sb4 in range(NSB // 4):
                    xq = xvqp.tile([128, MO, 512], F16, tag="xq")
                    nc.sync.dma_start(
                        out=xq[:, :, :], in_=xT16_v[:, :, sb4 * 512 : (sb4 + 1) * 512]
                    )
                    ps = psA.tile([128, S], F32, tag="psA")
                    for part in range(4):
                        for m in range(MO):
                            nc.tensor.matmul(
                                ps[:, part * 512 : part * 512 + 512],
                                xq[:, m, part * 128 : (part + 1) * 128],
                                wv[:, m, :],
                                start=(m == 0),
                                stop=(m == MO - 1),
                            )
                    nc.scalar.copy(v_sb[:, sb4 * 4 : sb4 * 4 + 4, :], ps[:, :])

            # ================= phase B: attention + out-projection =============
            with (
                tc.tile_pool(name="wop", bufs=1) as wop,
                tc.tile_pool(name="pp", bufs=6) as ppp,
                tc.tile_pool(name="pac", bufs=2) as pacp,
                tc.tile_pool(name="ao", bufs=2) as aop,
                tc.tile_pool(name="rcp", bufs=3) as rcp,
                tc.tile_pool(name="osb", bufs=3) as osbp,
                tc.tile_pool(name="mblk", bufs=4) as mblkp,
                tc.tile_pool(name="sp", bufs=2, space="PSUM") as spp,
                tc.tile_pool(name="acc", bufs=2, space="PSUM") as accp,
            ):
                wo = wop.tile([128, HPC, D_MODEL], F16, tag="wo")
                for oc in range(D_MODEL // 1024):
                    nc.sync.dma_start(
                        out=wo[:, :, oc * 1024 : (oc + 1) * 1024],
                        in_=woT_v[:, :, oc * 1024 : (oc + 1) * 1024],
                    )

                for qc in range(NQC):
                    q_lo = qc * 512
                    nki_here = (4 * qc + 4) if causal else NKI
                    aoT = aop.tile([128, HPC, 512], F16, tag="aoT")

                    # state for the lazily-emitted softmax denominator reduce
                    pending = []  # [(h, sav, pacc)]

                    def flush_sums():
                        if not pending:
                            return
                        h0, sav0, pacc0 = pending.pop()
                        nc.tensor.matmul(
                            sav0[:, 0:512],
                            ones_t[:, :],
                            pacc0[:, :],
                            start=True,
                            stop=True,
                        )
                        rc = rcp.tile([128, 512], F32, tag="rc")
                        nc.vector.reciprocal_approx_fast(rc[:, :], sav0[:, 0:512])
                        nc.vector.tensor_mul(
                            aoT[:, h0, :], sav0[:, 512:1024], rc[:, :]
                        )

                    for h in range(HPC):
                        sav = accp.tile([128, 1024], F32, tag="acc")
                        avp = sav[:, 512:1024]
                        pacc = pacp.tile([128, 512], F16, tag="pac")
                        n_full = 4 * qc if causal else nki_here
                        # full key blocks, two at a time (one wide Exp each)
                        for kp in range(n_full // 2):
                            ki0 = 2 * kp
                            spb2 = spp.tile([128, 1024], F32, tag="sp")
                            pp2 = ppp.tile([128, 1024], F16, tag="pp")
                            for j in range(2):
                                ki = ki0 + j
                                nc.tensor.matmul(
                                    spb2[:, j * 512 : (j + 1) * 512],
                                    k_rot[:, h, ki * 128 : (ki + 1) * 128],
                                    q_rot[:, h, q_lo : q_lo + 512],
                                    start=True,
                                    stop=True,
                                )
                            nc.scalar.activation(
                                pp2[:, :],
                                spb2[:, :],
                                mybir.ActivationFunctionType.Exp,
                                scale=float(exp_scale),
                            )
                            if not causal:
                                for j in range(2):
                                    ki = ki0 + j
                                    mb = mblkp.tile([128, 512], F16, tag="mblk")
                                    nc.sync.dma_start(
                                        out=mb[:, :],
                                        in_=maskf_v[:, ki, q_lo : q_lo + 512],
                                    )
                                    nc.vector.tensor_mul(
                                        pp2[:, j * 512 : (j + 1) * 512],
                                        pp2[:, j * 512 : (j + 1) * 512],
                                        mb[:, :],
                                    )
                            if ki0 == 0:
                                nc.vector.tensor_copy(pacc[:, :], pp2[:, 0:512])
                            else:
                                nc.vector.tensor_add(
                                    pacc[:, :], pacc[:, :], pp2[:, 0:512]
                                )
                            nc.vector.tensor_add(
                                pacc[:, :], pacc[:, :], pp2[:, 512:1024]
                            )
                            for j in range(2):
                                ki = ki0 + j
                                nc.tensor.matmul(
                                    avp[:, :],
                                    v_sb[:, ki, h * 128 : (h + 1) * 128],
                                    pp2[:, j * 512 : (j + 1) * 512],
                                    start=(ki == 0),
                                    stop=(ki == nki_here - 1),
                                )
                            if kp == 0:
                                flush_sums()
                        # diagonal key blocks (causal only), one at a time
                        for ki in range(n_full, nki_here):
                            q0 = 128 * (ki - 4 * qc)
                            spb = spp.tile([128, 1024], F32, tag="sp")
                            pp = ppp.tile([128, 512], F16, tag="pp")
                            nc.tensor.matmul(
                                spb[:, q0:512],
                                k_rot[:, h, ki * 128 : (ki + 1) * 128],
                                q_rot[:, h, q_lo + q0 : q_lo + 512],
                                start=True,
                                stop=True,
                            )
                            nc.scalar.activation(
                                pp[:, q0:512],
                                spb[:, q0:512],
                                mybir.ActivationFunctionType.Exp,
                                scale=float(exp_scale),
                            )
                            nc.vector.tensor_mul(
                                pp[:, q0 : q0 + 128],
                                pp[:, q0 : q0 + 128],
                                maskd[:, ki * 128 : (ki + 1) * 128],
                            )
                            if ki == 0:
                                nc.vector.tensor_copy(pacc[:, :], pp[:, :])
                            else:
                                nc.vector.tensor_add(
                                    pacc[:, q0:512], pacc[:, q0:512], pp[:, q0:512]
                                )
                            nc.tensor.matmul(
                                avp[:, q0:512],
                                v_sb[:, ki, h * 128 : (h + 1) * 128],
                                pp[:, q0:512],
                                start=(ki == 0),
                                stop=(ki == nki_here - 1),
                            )
                            if ki == n_full + 1:
                                flush_sums()
                        pending.append((h, sav, pacc))

                    # out-projection for this query chunk (4 seq blocks);
                    # h outer over oc pairs so each aoT stationary load
                    # serves two matmuls.  The last head's softmax reduce is
                    # flushed after the first three heads' first matmuls so
                    # the PE stays busy while the Vector engine finishes it.
                    for sb_l in range(4):
                        sb = 4 * qc + sb_l
                        ob = osbp.tile([128, 2048], F16, tag="osb")
                        for oc2 in range(2):
                            op2 = accp.tile([128, 1024], F32, tag="acc")
                            for h in range(HPC):
                                if pending and h == HPC - 1:
                                    flush_sums()
                                lhsT = aoT[:, h, sb_l * 128 : (sb_l + 1) * 128]
                                nc.tensor.matmul(
                                    op2[:, 0:512],
                                    lhsT,
                                    wo[:, h, (2 * oc2) * 512 : (2 * oc2 + 1) * 512],
                                    start=(h == 0),
                                    stop=(h == HPC - 1),
                                )
                                nc.tensor.matmul(
                                    op2[:, 512:1024],
                                    lhsT,
                                    wo[:, h, (2 * oc2 + 1) * 512 : (2 * oc2 + 2) * 512],
                                    start=(h == 0),
                                    stop=(h == HPC - 1),
                                )
                            nc.vector.tensor_copy(
                                ob[:, oc2 * 1024 : (oc2 + 1) * 1024], op2[:, :]
                            )
                        nc.sync.dma_start(
                            out=out_d[sb * 128 : (sb + 1) * 128, :],
                            in_=ob[:, :],
                        )

    nc.finalize()
    return nc


def _bit_quantize_ternary(w: np.ndarray):
    """Returns (ternary {-1,0,1} float32 matrix, scale) matching the reference."""
    scale = np.maximum(np.mean(np.abs(w.astype(np.float32))), np.float32(1e-5))
    t = np.clip(np.round(w.astype(np.float32) / scale), -1.0, 1.0).astype(np.float32)
    return t, float(scale)


def _host_tables():
    """cos/sin stacked [128, S]: rows 0:64 cos, rows 64:128 sin."""
    inv_freq = 1.0 / (ROPE_BASE ** (np.arange(0, D_HEAD, 2, dtype=np.float32) / D_HEAD))
    pos = np.arange(SEQ, dtype=np.float32)
    ang = pos[:, None] * inv_freq[None, :]  # [S, 64]
    cs = np.empty((128, SEQ), dtype=NPF16)
    cs[0:64] = np.ascontiguousarray(np.cos(ang).T).astype(NPF16)
    cs[64:128] = np.ascontiguousarray(np.sin(ang).T).astype(NPF16)
    sc = np.empty((128, SEQ), dtype=NPF16)
    sc[0:64] = cs[64:128]
    sc[64:128] = cs[0:64]
    return cs, sc


def kernel(x, w_qkv, w_out, mask):
    global LAST_RESULT
    x = np.asarray(x, dtype=np.float32)
    w_qkv = np.asarray(w_qkv, dtype=np.float32)
    w_out = np.asarray(w_out, dtype=np.float32)
    mask = np.asarray(mask)

    tq, sq = _bit_quantize_ternary(w_qkv)
    to, so = _bit_quantize_ternary(w_out)
    exp_scale = (sq * sq) / float(np.sqrt(D_HEAD))
    c2 = np.float32(sq * so)

    m2 = (mask.reshape(SEQ, SEQ) != 0).astype(np.float32)
    causal = bool(np.array_equal(m2, np.tril(np.ones((SEQ, SEQ), np.float32))))

    cs, sc = _host_tables()
    if causal:
        maskd = np.empty((128, SEQ), dtype=NPF16)
        for ki in range(NKI):
            blk = m2[ki * 128 : (ki + 1) * 128, ki * 128 : (ki + 1) * 128]  # [q, k]
            maskd[:, ki * 128 : (ki + 1) * 128] = np.ascontiguousarray(blk.T).astype(
                NPF16
            )
    else:
        maskf = np.ascontiguousarray(m2.T).astype(NPF16)  # [kk, qq]

    key = (causal, float(exp_scale))
    if key not in _PROG_CACHE:
        _PROG_CACHE[key] = _build_program(causal, float(exp_scale))
    nc = _PROG_CACHE[key]

    xT8 = [np.ascontiguousarray(x[b].T).astype(NPFP8) for b in range(BATCH)]
    xT16 = [np.ascontiguousarray(x[b].T).astype(NPF16) for b in range(BATCH)]

    in_maps = []
    for c in range(N_CORES):
        b, g = divmod(c, 4)
        rows = slice(R_LOCAL * g, R_LOCAL * (g + 1))
        im = {
            "xT8": xT8[b],
            "xT16": xT16[b],
            "wqT": np.ascontiguousarray(tq[0 * D_MODEL :][rows].T).astype(NPFP8),
            "wkT": np.ascontiguousarray(tq[1 * D_MODEL :][rows].T).astype(NPFP8),
            "wvT": np.ascontiguousarray(tq[2 * D_MODEL :][rows].T).astype(NPF16),
            "woT": np.ascontiguousarray(to[:, rows].T).astype(NPF16),
            "cossinT": cs,
            "sincosT": sc,
        }
        if causal:
            im["maskd"] = maskd
        else:
            im["maskf"] = maskf
        in_maps.append(im)

    do_trace = bool(PROFILE) and _enable_profiling()
    res = run_bass_kernel_spmd(nc, in_maps, list(range(N_CORES)), trace=do_trace)
    LAST_RESULT = res

    parts = [np.asarray(res.results[c]["out"]).astype(np.float32) for c in range(N_CORES)]
    out = np.stack(
        [
            parts[0] + parts[1] + parts[2] + parts[3],
            parts[4] + parts[5] + parts[6] + parts[7],
        ]
    )
    return (out * c2).astype(np.float32)


# revision 14
# speedup vs baseline: 1.1802x; 1.0160x over previous
"""Trainium2 Bass kernel for BitNet multi-head attention (nn_MultiHeadAttention_62294205661880).

Sharding: 8 cores = 2 batches x 4 head-groups (4 heads each).  Each core
computes qkv projection, RoPE, causal attention and a column-parallel slice
of the output projection for its (batch, head-group); the host sums the 4
partial out-projections per batch (the tensor-parallel all-reduce done
host-side, since the contract gathers to host anyway).

BitNet quantization is folded on the host: weights are uploaded as exact
ternary {-1,0,+1} matrices; scale_qkv^2/sqrt(dh) is folded into the
softmax exp() scale and scale_qkv*scale_out into a final host-side scalar.

Precision/speed split:
 - Q/K projections run as fp8e4m3 DoubleRow matmuls (2 contraction blocks
   per pass): x and the exact-ternary weights are fp8.  The fp8 noise on
   q/k is softened by softmax normalization (measured ~1e-2 rel err).
 - Everything else (V path, scores, attention, out-projection) runs in
   fp16 (same PE speed as bf16, 8x the mantissa) to keep margin.
 - The softmax denominator is accumulated across key blocks on the Vector
   engine (fp16 adds) and reduced over partitions with a single ones-
   matmul per (query chunk, head) instead of one matmul per key block.
   That ones-matmul is emitted lazily (inside the NEXT head's block loop)
   so the PE never waits on the Vector engine's accumulator chain.
 - Full (non-diagonal) key blocks are processed in pairs so each Exp
   activation covers 1024 columns (the ACT engine would otherwise pace
   the attention inner loop).

Device layout trick: everything is computed transposed.  Q_T/K_T come out of
the projection as [dh, S]; scores are computed as s_T[k, q]; AV produces
out_T[dh, q] which feeds the output projection directly.  No on-device
transposes at all.  Softmax skips the max-subtraction: scores are bounded
(~+-2) because the BitNet weight scale is tiny, so exp() is safe.
"""

import sys
import types

import numpy as np
import ml_dtypes

import concourse.bass as bass
import concourse.mybir as mybir
import concourse.tile as tile
from concourse import bacc
from concourse.bass_utils import run_bass_kernel_spmd

D_MODEL = 2048
N_HEADS = 16
D_HEAD = 128
SEQ = 2048
BATCH = 2
ROPE_BASE = 10000.0

N_CORES = 8
HPC = 4  # heads per core
R_LOCAL = HPC * D_HEAD  # 512 local q (or k, or v) rows per core
MO = D_MODEL // 128  # 16 contraction blocks
MP = MO // 2  # 8 contraction block pairs (DoubleRow)
NKI = SEQ // 128  # 16 key blocks
NQC = SEQ // 512  # 4 query chunks of 512
NSB = SEQ // 128  # 16 seq blocks (v / proj)
N_WARM = 80  # PE warm-up matmuls issued while the first DMAs land

F16 = mybir.dt.float16
F32 = mybir.dt.float32
NPF16 = np.float16
NPFP8 = ml_dtypes.float8_e4m3
FP8 = mybir.dt.float8e4
DR = mybir.MatmulPerfMode.DoubleRow

LAST_RESULT = None  # BassKernelResults of the most recent run (for test.py)
_PROG_CACHE = {}
PROFILE = False  # test.py sets True to capture an NTFF profile / HW exec time


def _enable_profiling() -> bool:
    """Install the axon NTFF profile hook glue if the image lacks
    ``antenv.axon_hooks`` (boot degrades silently without it), and skip
    the artifact upload (no bucket access in this container)."""
    try:
        from antenv.axon_hooks import get_axon_ntff_profile_hook  # noqa: F401

        ok = get_axon_ntff_profile_hook() is not None
    except ImportError:
        ok = False
        import antenv

        mod = types.ModuleType("antenv.axon_hooks")
        mod._hook = None
        mod.set_axon_ntff_profile_hook = lambda h: setattr(mod, "_hook", h)
        mod.get_axon_ntff_profile_hook = lambda: mod._hook
        sys.modules["antenv.axon_hooks"] = mod
        antenv.axon_hooks = mod
        try:
            from trn_agent_boot.trn_boot import _ntff_profile_via_ctypes

            hook = _ntff_profile_via_ctypes("/opt/axon/libaxon_pjrt.so")
            if hook is not None:
                mod._hook = hook
                ok = True
        except Exception as e:  # profiling is best-effort
            print(f"ntff profile hook install failed: {e}", file=sys.stderr)
    if ok:
        import concourse.bass_utils as _bu

        _bu.upload_artifacts = lambda tmpdir: tmpdir
    return ok


def _build_program(causal: bool, exp_scale: float) -> bass.Bass:
    nc = bacc.Bacc(None)
    S = SEQ

    # weights / x are pre-swizzled on the host so every DMA reads large
    # contiguous per-partition spans (8-16KB descriptors instead of 512B)
    xT8_d = nc.dram_tensor("xT8", [D_MODEL, S], FP8, kind="ExternalInput")
    xQ16_d = nc.dram_tensor("xQ16", [4 * 128, MO * 512], F16, kind="ExternalInput")
    wqS_d = nc.dram_tensor("wqS", [128, MO * R_LOCAL], FP8, kind="ExternalInput")
    wkS_d = nc.dram_tensor("wkS", [128, MO * R_LOCAL], FP8, kind="ExternalInput")
    wvS_d = nc.dram_tensor("wvS", [128, MO * R_LOCAL], F16, kind="ExternalInput")
    woS_d = nc.dram_tensor("woS", [128, HPC * D_MODEL], F16, kind="ExternalInput")
    # cos rows 0:64, sin rows 64:128
    cs_d = nc.dram_tensor("cossinT", [128, S], F16, kind="ExternalInput")
    # swapped: sin rows 0:64, cos rows 64:128 (keeps TensorTensor base partitions equal)
    sc_d = nc.dram_tensor("sincosT", [128, S], F16, kind="ExternalInput")
    if causal:
        # 16 transposed diagonal 128x128 mask blocks, side by side
        maskd_d = nc.dram_tensor("maskd", [128, S], F16, kind="ExternalInput")
    else:
        maskf_d = nc.dram_tensor("maskf", [S, S], F16, kind="ExternalInput")
    out_d = nc.dram_tensor("out", [S, D_MODEL], F16, kind="ExternalOutput")

    xT8_v = xT8_d[:].rearrange("(mo p) s -> p mo s", p=128)
    xQ16_v = xQ16_d[:].rearrange("(q p) (mo s) -> p q mo s", p=128, mo=MO)
    wqS_v = wqS_d[:].rearrange("p (mo r) -> p mo r", mo=MO)
    wkS_v = wkS_d[:].rearrange("p (mo r) -> p mo r", mo=MO)
    wvS_v = wvS_d[:].rearrange("p (mo r) -> p mo r", mo=MO)
    woS_v = woS_d[:].rearrange("p (h o) -> p h o", h=HPC)
    if not causal:
        maskf_v = maskf_d[:].rearrange("(ko p) q -> p ko q", p=128)

    with tile.TileContext(nc) as tc:
        with tc.tile_pool(name="pers", bufs=1) as pers:
            # ---- persistent SBUF tensors (live across both phases) ----
            q_rot = pers.tile([128, HPC, S], F16, tag="qrot")
            k_rot = pers.tile([128, HPC, S], F16, tag="krot")
            v_sb = pers.tile([128, NKI, R_LOCAL], F16, tag="vsb")
            ones_t = pers.tile([128, 128], F16, tag="ones")
            warm = pers.tile([128, 1], F16, tag="warm")
            if causal:
                maskd = pers.tile([128, S], F16, tag="maskd")
            nc.vector.memset(ones_t[:, :], 1.0)
            # load the exp table set first so no ACT table switch happens
            # mid-kernel (Copy lives in every set).
            nc.scalar.activation(
                warm[:, :], ones_t[:, 0:1], mybir.ActivationFunctionType.Exp
            )

            # ================= phase A: QKV projection + RoPE =================
            with (
                tc.tile_pool(name="xtp", bufs=1) as xtp,
                tc.tile_pool(name="xvq", bufs=2) as xvqp,
                tc.tile_pool(name="wp", bufs=1) as wp,
                tc.tile_pool(name="raw", bufs=2) as rawp,
                tc.tile_pool(name="tmp", bufs=2) as tmpp,
                tc.tile_pool(name="psA", bufs=2, space="PSUM") as psA,
            ):
                xt8 = xtp.tile([128, MO, S], FP8, tag="xt8")
                wq = wp.tile([128, MO, R_LOCAL], FP8, tag="wq")
                wk = wp.tile([128, MO, R_LOCAL], FP8, tag="wk")
                wv = wp.tile([128, MO, R_LOCAL], F16, tag="wv")
                cs_t = wp.tile([128, S], F16, tag="cs")
                sc_t = wp.tile([128, S], F16, tag="sc")

                # warm up the PE / HAM while the first input DMAs land
                warmps = psA.tile([128, 128], F32, tag="psA")
                for _ in range(N_WARM):
                    nc.tensor.matmul(
                        warmps[:, :],
                        ones_t[:, :],
                        ones_t[:, :],
                        start=True,
                        stop=True,
                    )

                # DMA order: the first m-pairs' weights/x first (they gate the
                # first real matmul), then the rest in large contiguous spans.
                nc.sync.dma_start(out=wq[:, 0:4, :], in_=wqS_v[:, 0:4, :])
                nc.sync.dma_start(out=wk[:, 0:4, :], in_=wkS_v[:, 0:4, :])
                for mo in range(4):
                    nc.sync.dma_start(out=xt8[:, mo, :], in_=xT8_v[:, mo, :])
                nc.sync.dma_start(out=wq[:, 4:16, :], in_=wqS_v[:, 4:16, :])
                nc.sync.dma_start(out=wk[:, 4:16, :], in_=wkS_v[:, 4:16, :])
                for g4 in range(1, 4):
                    sl = slice(4 * g4, 4 * g4 + 4)
                    nc.sync.dma_start(out=xt8[:, sl, :], in_=xT8_v[:, sl, :])
                nc.sync.dma_start(out=cs_t[:, :], in_=cs_d[:, :])
                nc.sync.dma_start(out=sc_t[:, :], in_=sc_d[:, :])
                if causal:
                    nc.sync.dma_start(out=maskd[:, :], in_=maskd_d[:, :])
                nc.sync.dma_start(out=wv[:, :, :], in_=wvS_v[:, :, :])

                def rope(dst, raw):
                    """NeoX rotary: rows 0:64 = t*c - b*s ; rows 64:128 = t*s + b*c."""
                    ta = tmpp.tile([64, S], F16, tag="tmp")
                    tb = tmpp.tile([64, S], F16, tag="tmp")
                    nc.vector.tensor_mul(ta[:, :], raw[0:64, :], cs_t[0:64, :])
                    nc.vector.tensor_mul(tb[:, :], raw[64:128, :], cs_t[64:128, :])
                    nc.vector.tensor_sub(dst[0:64, :], ta[:, :], tb[:, :])
                    tc2 = tmpp.tile([64, S], F16, tag="tmp")
                    td = tmpp.tile([64, S], F16, tag="tmp")
                    nc.vector.tensor_mul(tc2[:, :], raw[0:64, :], sc_t[0:64, :])
                    nc.vector.tensor_mul(td[:, :], raw[64:128, :], sc_t[64:128, :])
                    nc.vector.tensor_add(dst[64:128, :], tc2[:, :], td[:, :])

                # head 0 q/k with the m-pair loop OUTER so the matmuls consume
                # xt8 m-blocks as the DMAs land (startup overlap).
                qp0 = psA.tile([128, S], F32, tag="psA")
                kp0 = psA.tile([128, S], F32, tag="psA")
                for mp in range(MP):
                    for c4 in range(4):
                        nc.tensor.matmul(
                            qp0[:, c4 * 512 : (c4 + 1) * 512],
                            wq[:, 2 * mp : 2 * mp + 2, 0:128],
                            xt8[:, 2 * mp : 2 * mp + 2, c4 * 512 : (c4 + 1) * 512],
                            start=(mp == 0),
                            stop=(mp == MP - 1),
                            perf_mode=DR,
                        )
                        nc.tensor.matmul(
                            kp0[:, c4 * 512 : (c4 + 1) * 512],
                            wk[:, 2 * mp : 2 * mp + 2, 0:128],
                            xt8[:, 2 * mp : 2 * mp + 2, c4 * 512 : (c4 + 1) * 512],
                            start=(mp == 0),
                            stop=(mp == MP - 1),
                            perf_mode=DR,
                        )
                q_raw = rawp.tile([128, S], F16, tag="raw")
                nc.scalar.copy(q_raw[:, :], qp0[:, :])
                rope(q_rot[:, 0, :], q_raw)
                k_raw = rawp.tile([128, S], F16, tag="raw")
                nc.scalar.copy(k_raw[:, :], kp0[:, :])
                rope(k_rot[:, 0, :], k_raw)

                def project(dst_raw, w_sb, h):
                    """q/k head projection -> fp16 raw [128, S] (xt8 resident)."""
                    ps = psA.tile([128, S], F32, tag="psA")
                    for c4 in range(4):
                        for mp in range(MP):
                            nc.tensor.matmul(
                                ps[:, c4 * 512 : (c4 + 1) * 512],
                                w_sb[:, 2 * mp : 2 * mp + 2, h * 128 : (h + 1) * 128],
                                xt8[:, 2 * mp : 2 * mp + 2, c4 * 512 : (c4 + 1) * 512],
                                start=(mp == 0),
                                stop=(mp == MP - 1),
                                perf_mode=DR,
                            )
                    nc.scalar.copy(dst_raw[:, :], ps[:, :])

                for h in range(1, HPC):
                    q_raw = rawp.tile([128, S], F16, tag="raw")
                    project(q_raw, wq, h)
                    rope(q_rot[:, h, :], q_raw)
                    k_raw = rawp.tile([128, S], F16, tag="raw")
                    project(k_raw, wk, h)
                    rope(k_rot[:, h, :], k_raw)

                # V projection (natural layout [s, r]) in fp16; x quarters are
                # streamed through a double-buffered pool (SBUF can't hold the
                # fp16 x alongside the fp8 copy).
                for sb4 in range(NSB // 4):
                    xq = xvqp.tile([128, MO, 512], F16, tag="xq")
                    nc.sync.dma_start(out=xq[:, :, :], in_=xQ16_v[:, sb4, :, :])
                    ps = psA.tile([128, S], F32, tag="psA")
                    for part in range(4):
                        for m in range(MO):
                            nc.tensor.matmul(
                                ps[:, part * 512 : part * 512 + 512],
                                xq[:, m, part * 128 : (part + 1) * 128],
                                wv[:, m, :],
                                start=(m == 0),
                                stop=(m == MO - 1),
                            )
                    nc.scalar.copy(v_sb[:, sb4 * 4 : sb4 * 4 + 4, :], ps[:, :])

            # ================= phase B: attention + out-projection =============
            with (
                tc.tile_pool(name="wop", bufs=1) as wop,
                tc.tile_pool(name="pp", bufs=6) as ppp,
                tc.tile_pool(name="pac", bufs=2) as pacp,
                tc.tile_pool(name="ao", bufs=2) as aop,
                tc.tile_pool(name="rcp", bufs=3) as rcp,
                tc.tile_pool(name="osb", bufs=3) as osbp,
                tc.tile_pool(name="mblk", bufs=4) as mblkp,
                tc.tile_pool(name="sp", bufs=2, space="PSUM") as spp,
                tc.tile_pool(name="acc", bufs=2, space="PSUM") as accp,
            ):
                wo = wop.tile([128, HPC, D_MODEL], F16, tag="wo")
                nc.sync.dma_start(out=wo[:, :, :], in_=woS_v[:, :, :])

                for qc in range(NQC):
                    q_lo = qc * 512
                    nki_here = (4 * qc + 4) if causal else NKI
                    aoT = aop.tile([128, HPC, 512], F16, tag="aoT")

                    # state for the lazily-emitted softmax denominator reduce
                    pending = []  # [(h, sav, pacc, split)]

                    def flush_sums():
                        if not pending:
                            return
                        h0, sav0, pacc0, split0 = pending.pop()
                        if split0:
                            # fold the right half of the wide accumulator in
                            nc.vector.tensor_add(
                                pacc0[:, 0:512], pacc0[:, 0:512], pacc0[:, 512:1024]
                            )
                        nc.tensor.matmul(
                            sav0[:, 0:512],
                            ones_t[:, :],
                            pacc0[:, 0:512],
                            start=True,
                            stop=True,
                        )
                        rc = rcp.tile([128, 512], F32, tag="rc")
                        nc.vector.reciprocal_approx_fast(rc[:, :], sav0[:, 0:512])
                        nc.vector.tensor_mul(
                            aoT[:, h0, :], sav0[:, 512:1024], rc[:, :]
                        )

                    for h in range(HPC):
                        sav = accp.tile([128, 1024], F32, tag="acc")
                        avp = sav[:, 512:1024]
                        pacc = pacp.tile([128, 1024], F16, tag="pac")
                        n_full = 4 * qc if causal else nki_here
                        # full key blocks, two at a time (one wide Exp each)
                        for kp in range(n_full // 2):
                            ki0 = 2 * kp
                            spb2 = spp.tile([128, 1024], F32, tag="sp")
                            pp2 = ppp.tile([128, 1024], F16, tag="pp")
                            for j in range(2):
                                ki = ki0 + j
                                nc.tensor.matmul(
                                    spb2[:, j * 512 : (j + 1) * 512],
                                    k_rot[:, h, ki * 128 : (ki + 1) * 128],
                                    q_rot[:, h, q_lo : q_lo + 512],
                                    start=True,
                                    stop=True,
                                )
                            nc.scalar.activation(
                                pp2[:, :],
                                spb2[:, :],
                                mybir.ActivationFunctionType.Exp,
                                scale=float(exp_scale),
                            )
                            if not causal:
                                for j in range(2):
                                    ki = ki0 + j
                                    mb = mblkp.tile([128, 512], F16, tag="mblk")
                                    nc.sync.dma_start(
                                        out=mb[:, :],
                                        in_=maskf_v[:, ki, q_lo : q_lo + 512],
                                    )
                                    nc.vector.tensor_mul(
                                        pp2[:, j * 512 : (j + 1) * 512],
                                        pp2[:, j * 512 : (j + 1) * 512],
                                        mb[:, :],
                                    )
                            if ki0 == 0:
                                nc.vector.tensor_copy(pacc[:, :], pp2[:, :])
                            else:
                                nc.vector.tensor_add(pacc[:, :], pacc[:, :], pp2[:, :])
                            for j in range(2):
                                ki = ki0 + j
                                nc.tensor.matmul(
                                    avp[:, :],
                                    v_sb[:, ki, h * 128 : (h + 1) * 128],
                                    pp2[:, j * 512 : (j + 1) * 512],
                                    start=(ki == 0),
                                    stop=(ki == nki_here - 1),
                                )
                            if kp == 0:
                                flush_sums()
                        # diagonal key blocks (causal only), two per Exp: the
                        # pair shares one [128,1024] score tile; the not-
                        # computed corners hold stale PSUM, exp() of which is
                        # written to pp slots nothing ever reads.
                        for kd in range((nki_here - n_full) // 2):
                            kia = n_full + 2 * kd
                            q0a = 128 * (kia - 4 * qc)
                            q0b = q0a + 128
                            spb2 = spp.tile([128, 1024], F32, tag="sp")
                            pp2 = ppp.tile([128, 1024], F16, tag="pp")
                            for j, q0 in ((0, q0a), (1, q0b)):
                                ki = kia + j
                                nc.tensor.matmul(
                                    spb2[:, j * 512 + q0 : (j + 1) * 512],
                                    k_rot[:, h, ki * 128 : (ki + 1) * 128],
                                    q_rot[:, h, q_lo + q0 : q_lo + 512],
                                    start=True,
                                    stop=True,
                                )
                            nc.scalar.activation(
                                pp2[:, :],
                                spb2[:, :],
                                mybir.ActivationFunctionType.Exp,
                                scale=float(exp_scale),
                            )
                            for j, q0 in ((0, q0a), (1, q0b)):
                                ki = kia + j
                                nc.vector.tensor_mul(
                                    pp2[:, j * 512 + q0 : j * 512 + q0 + 128],
                                    pp2[:, j * 512 + q0 : j * 512 + q0 + 128],
                                    maskd[:, ki * 128 : (ki + 1) * 128],
                                )
                            if kia == 0:
                                nc.vector.tensor_copy(pacc[:, 0:512], pp2[:, 0:512])
                            else:
                                nc.vector.tensor_add(
                                    pacc[:, q0a:512], pacc[:, q0a:512], pp2[:, q0a:512]
                                )
                            nc.vector.tensor_add(
                                pacc[:, q0b:512],
                                pacc[:, q0b:512],
                                pp2[:, 512 + q0b : 1024],
                            )
                            for j, q0 in ((0, q0a), (1, q0b)):
                                ki = kia + j
                                nc.tensor.matmul(
                                    avp[:, q0:512],
                                    v_sb[:, ki, h * 128 : (h + 1) * 128],
                                    pp2[:, j * 512 + q0 : (j + 1) * 512],
                                    start=(ki == 0),
                                    stop=(ki == nki_here - 1),
                                )
                            if kd == 0:
                                flush_sums()
                        pending.append((h, sav, pacc, n_full > 0))

                    # out-projection for this query chunk (4 seq blocks);
                    # h outer over oc pairs so each aoT stationary load
                    # serves two matmuls.  The last head's softmax reduce is
                    # flushed after the first three heads' first matmuls so
                    # the PE stays busy while the Vector engine finishes it.
                    for sb_l in range(4):
                        sb = 4 * qc + sb_l
                        ob = osbp.tile([128, 2048], F16, tag="osb")
                        for oc2 in range(2):
                            op2 = accp.tile([128, 1024], F32, tag="acc")
                            for h in range(HPC):
                                if pending and h == HPC - 1:
                                    flush_sums()
                                lhsT = aoT[:, h, sb_l * 128 : (sb_l + 1) * 128]
                                nc.tensor.matmul(
                                    op2[:, 0:512],
                                    lhsT,
                                    wo[:, h, (2 * oc2) * 512 : (2 * oc2 + 1) * 512],
                                    start=(h == 0),
                                    stop=(h == HPC - 1),
                                )
                                nc.tensor.matmul(
                                    op2[:, 512:1024],
                                    lhsT,
                                    wo[:, h, (2 * oc2 + 1) * 512 : (2 * oc2 + 2) * 512],
                                    start=(h == 0),
                                    stop=(h == HPC - 1),
                                )
                            dst = ob[:, oc2 * 1024 : (oc2 + 1) * 1024]
                            if oc2 == 0:
                                nc.vector.tensor_copy(dst, op2[:, :])
                            else:
                                nc.scalar.copy(dst, op2[:, :])
                        nc.sync.dma_start(
                            out=out_d[sb * 128 : (sb + 1) * 128, :],
                            in_=ob[:, :],
                        )

    nc.finalize()
    return nc


def _bit_quantize_ternary(w: np.ndarray):
    """Returns (ternary {-1,0,1} float32 matrix, scale) matching the reference."""
    scale = np.maximum(np.mean(np.abs(w.astype(np.float32))), np.float32(1e-5))
    t = np.clip(np.round(w.astype(np.float32) / scale), -1.0, 1.0).astype(np.float32)
    return t, float(scale)


def _host_tables():
    """cos/sin stacked [128, S]: rows 0:64 cos, rows 64:128 sin."""
    inv_freq = 1.0 / (ROPE_BASE ** (np.arange(0, D_HEAD, 2, dtype=np.float32) / D_HEAD))
    pos = np.arange(SEQ, dtype=np.float32)
    ang = pos[:, None] * inv_freq[None, :]  # [S, 64]
    cs = np.empty((128, SEQ), dtype=NPF16)
    cs[0:64] = np.ascontiguousarray(np.cos(ang).T).astype(NPF16)
    cs[64:128] = np.ascontiguousarray(np.sin(ang).T).astype(NPF16)
    sc = np.empty((128, SEQ), dtype=NPF16)
    sc[0:64] = cs[64:128]
    sc[64:128] = cs[0:64]
    return cs, sc


def kernel(x, w_qkv, w_out, mask):
    global LAST_RESULT
    x = np.asarray(x, dtype=np.float32)
    w_qkv = np.asarray(w_qkv, dtype=np.float32)
    w_out = np.asarray(w_out, dtype=np.float32)
    mask = np.asarray(mask)

    tq, sq = _bit_quantize_ternary(w_qkv)
    to, so = _bit_quantize_ternary(w_out)
    exp_scale = (sq * sq) / float(np.sqrt(D_HEAD))
    c2 = np.float32(sq * so)

    m2 = (mask.reshape(SEQ, SEQ) != 0).astype(np.float32)
    causal = bool(np.array_equal(m2, np.tril(np.ones((SEQ, SEQ), np.float32))))

    cs, sc = _host_tables()
    if causal:
        maskd = np.empty((128, SEQ), dtype=NPF16)
        for ki in range(NKI):
            blk = m2[ki * 128 : (ki + 1) * 128, ki * 128 : (ki + 1) * 128]  # [q, k]
            maskd[:, ki * 128 : (ki + 1) * 128] = np.ascontiguousarray(blk.T).astype(
                NPF16
            )
    else:
        maskf = np.ascontiguousarray(m2.T).astype(NPF16)  # [kk, qq]

    key = (causal, float(exp_scale))
    if key not in _PROG_CACHE:
        _PROG_CACHE[key] = _build_program(causal, float(exp_scale))
    nc = _PROG_CACHE[key]

    def _swz_w(wT):  # [D_MODEL, R_LOCAL] -> [128, MO*R_LOCAL] partition-major
        return np.ascontiguousarray(
            wT.reshape(MO, 128, R_LOCAL).transpose(1, 0, 2).reshape(128, MO * R_LOCAL)
        )

    xT8 = [np.ascontiguousarray(x[b].T).astype(NPFP8) for b in range(BATCH)]
    xQ16 = [
        np.ascontiguousarray(
            x[b].T.reshape(MO, 128, 4, 512)
            .transpose(2, 1, 0, 3)
            .reshape(4 * 128, MO * 512)
        ).astype(NPF16)
        for b in range(BATCH)
    ]

    in_maps = []
    for c in range(N_CORES):
        b, g = divmod(c, 4)
        rows = slice(R_LOCAL * g, R_LOCAL * (g + 1))
        woT = to[:, rows].T  # [R_LOCAL, D_MODEL]
        woS = (
            woT.reshape(HPC, 128, D_MODEL)
            .transpose(1, 0, 2)
            .reshape(128, HPC * D_MODEL)
        )
        im = {
            "xT8": xT8[b],
            "xQ16": xQ16[b],
            "wqS": _swz_w(tq[0 * D_MODEL :][rows].T).astype(NPFP8),
            "wkS": _swz_w(tq[1 * D_MODEL :][rows].T).astype(NPFP8),
            "wvS": _swz_w(tq[2 * D_MODEL :][rows].T).astype(NPF16),
            "woS": np.ascontiguousarray(woS).astype(NPF16),
            "cossinT": cs,
            "sincosT": sc,
        }
        if causal:
            im["maskd"] = maskd
        else:
            im["maskf"] = maskf
        in_maps.append(im)

    do_trace = bool(PROFILE) and _enable_profiling()
    res = run_bass_kernel_spmd(nc, in_maps, list(range(N_CORES)), trace=do_trace)
    LAST_RESULT = res

    parts = [np.asarray(res.results[c]["out"]).astype(np.float32) for c in range(N_CORES)]
    out = np.stack(
        [
            parts[0] + parts[1] + parts[2] + parts[3],
            parts[4] + parts[5] + parts[6] + parts[7],
        ]
    )
    return (out * c2).astype(np.float32)


# revision 18
# speedup vs baseline: 1.2227x; 1.0361x over previous
"""Trainium2 Bass kernel for BitNet multi-head attention (nn_MultiHeadAttention_62294205661880).

Sharding: 8 cores = 2 batches x 4 head-groups (4 heads each).  Each core
computes qkv projection, RoPE, causal attention and a column-parallel slice
of the output projection for its (batch, head-group); the host sums the 4
partial out-projections per batch (the tensor-parallel all-reduce done
host-side, since the contract gathers to host anyway).

BitNet quantization is folded on the host: weights are uploaded as exact
ternary {-1,0,+1} matrices; scale_qkv^2/sqrt(dh) is folded into the
softmax exp() scale and scale_qkv*scale_out into a final host-side scalar.

Precision/speed split:
 - Q/K projections run as fp8e4m3 DoubleRow matmuls (2 contraction blocks
   per pass): x and the exact-ternary weights are fp8.  The fp8 noise on
   q/k is softened by softmax normalization (measured ~1e-2 rel err).
 - Everything else (V path, scores, attention, out-projection) runs in
   fp16 (same PE speed as bf16, 8x the mantissa) to keep margin.
 - The softmax denominator is accumulated across key blocks on the Vector
   engine (fp16 adds) and reduced over partitions with a single ones-
   matmul per (query chunk, head) instead of one matmul per key block.
   That ones-matmul is emitted lazily (inside the NEXT head's block loop)
   so the PE never waits on the Vector engine's accumulator chain.
 - Full (non-diagonal) key blocks are processed in pairs so each Exp
   activation covers 1024 columns (the ACT engine would otherwise pace
   the attention inner loop).

Device layout trick: everything is computed transposed.  Q_T/K_T come out of
the projection as [dh, S]; scores are computed as s_T[k, q]; AV produces
out_T[dh, q] which feeds the output projection directly.  No on-device
transposes at all.  Softmax skips the max-subtraction: scores are bounded
(~+-2) because the BitNet weight scale is tiny, so exp() is safe.
"""

import sys
import types

import numpy as np
import ml_dtypes

import concourse.bass as bass
import concourse.mybir as mybir
import concourse.tile as tile
from concourse import bacc
from concourse.bass_utils import run_bass_kernel_spmd

D_MODEL = 2048
N_HEADS = 16
D_HEAD = 128
SEQ = 2048
BATCH = 2
ROPE_BASE = 10000.0

N_CORES = 8
HPC = 4  # heads per core
R_LOCAL = HPC * D_HEAD  # 512 local q (or k, or v) rows per core
MO = D_MODEL // 128  # 16 contraction blocks
MP = MO // 2  # 8 contraction block pairs (DoubleRow)
NKI = SEQ // 128  # 16 key blocks
NQC = SEQ // 512  # 4 query chunks of 512
NSB = SEQ // 128  # 16 seq blocks (v / proj)
N_WARM = 55  # PE warm-up matmuls issued while the first DMAs land

F16 = mybir.dt.float16
F32 = mybir.dt.float32
NPF16 = np.float16
NPFP8 = ml_dtypes.float8_e4m3
FP8 = mybir.dt.float8e4
DR = mybir.MatmulPerfMode.DoubleRow

LAST_RESULT = None  # BassKernelResults of the most recent run (for test.py)
_PROG_CACHE = {}
PROFILE = False  # test.py sets True to capture an NTFF profile / HW exec time


def _enable_profiling() -> bool:
    """Install the axon NTFF profile hook glue if the image lacks
    ``antenv.axon_hooks`` (boot degrades silently without it), and skip
    the artifact upload (no bucket access in this container)."""
    try:
        from antenv.axon_hooks import get_axon_ntff_profile_hook  # noqa: F401

        ok = get_axon_ntff_profile_hook() is not None
    except ImportError:
        ok = False
        import antenv

        mod = types.ModuleType("antenv.axon_hooks")
        mod._hook = None
        mod.set_axon_ntff_profile_hook = lambda h: setattr(mod, "_hook", h)
        mod.get_axon_ntff_profile_hook = lambda: mod._hook
        sys.modules["antenv.axon_hooks"] = mod
        antenv.axon_hooks = mod
        try:
            from trn_agent_boot.trn_boot import _ntff_profile_via_ctypes

            hook = _ntff_profile_via_ctypes("/opt/axon/libaxon_pjrt.so")
            if hook is not None:
                mod._hook = hook
                ok = True
        except Exception as e:  # profiling is best-effort
            print(f"ntff profile hook install failed: {e}", file=sys.stderr)
    if ok:
        import concourse.bass_utils as _bu

        _bu.upload_artifacts = lambda tmpdir: tmpdir
    return ok


def _build_program(causal: bool, exp_scale: float) -> bass.Bass:
    nc = bacc.Bacc(None)
    S = SEQ

    # weights / x are pre-swizzled on the host so every DMA reads large
    # contiguous per-partition spans (8-16KB descriptors instead of 512B)
    xT8_d = nc.dram_tensor("xT8", [D_MODEL, S], FP8, kind="ExternalInput")
    xQ16_d = nc.dram_tensor("xQ16", [4 * 128, MO * 512], F16, kind="ExternalInput")
    wqS_d = nc.dram_tensor("wqS", [128, MO * R_LOCAL], FP8, kind="ExternalInput")
    wkS_d = nc.dram_tensor("wkS", [128, MO * R_LOCAL], FP8, kind="ExternalInput")
    wvS_d = nc.dram_tensor("wvS", [128, MO * R_LOCAL], F16, kind="ExternalInput")
    woS_d = nc.dram_tensor("woS", [128, HPC * D_MODEL], F16, kind="ExternalInput")
    # cos rows 0:64, sin rows 64:128
    cs_d = nc.dram_tensor("cossinT", [128, S], F16, kind="ExternalInput")
    # swapped: sin rows 0:64, cos rows 64:128 (keeps TensorTensor base partitions equal)
    sc_d = nc.dram_tensor("sincosT", [128, S], F16, kind="ExternalInput")
    if causal:
        # 16 transposed diagonal 128x128 mask blocks, side by side
        maskd_d = nc.dram_tensor("maskd", [128, S], F16, kind="ExternalInput")
    else:
        maskf_d = nc.dram_tensor("maskf", [S, S], F16, kind="ExternalInput")
    out_d = nc.dram_tensor("out", [S, D_MODEL], F16, kind="ExternalOutput")

    xT8_v = xT8_d[:].rearrange("(mo p) s -> p mo s", p=128)
    xQ16_v = xQ16_d[:].rearrange("(q p) (mo s) -> p q mo s", p=128, mo=MO)
    wqS_v = wqS_d[:].rearrange("p (mo r) -> p mo r", mo=MO)
    wkS_v = wkS_d[:].rearrange("p (mo r) -> p mo r", mo=MO)
    wvS_v = wvS_d[:].rearrange("p (mo r) -> p mo r", mo=MO)
    woS_v = woS_d[:].rearrange("p (h o) -> p h o", h=HPC)
    if not causal:
        maskf_v = maskf_d[:].rearrange("(ko p) q -> p ko q", p=128)

    with tile.TileContext(nc) as tc:
        with tc.tile_pool(name="pers", bufs=1) as pers:
            # ---- persistent SBUF tensors (live across both phases) ----
            q_rot = pers.tile([128, HPC, S], F16, tag="qrot")
            k_rot = pers.tile([128, HPC, S], F16, tag="krot")
            v_sb = pers.tile([128, NKI, R_LOCAL], F16, tag="vsb")
            ones_t = pers.tile([128, 128], F16, tag="ones")
            warm = pers.tile([128, 1], F16, tag="warm")
            if causal:
                maskd = pers.tile([128, S], F16, tag="maskd")
            nc.vector.memset(ones_t[:, :], 1.0)
            # load the exp table set first so no ACT table switch happens
            # mid-kernel (Copy lives in every set).
            nc.scalar.activation(
                warm[:, :], ones_t[:, 0:1], mybir.ActivationFunctionType.Exp
            )

            # ================= phase A: QKV projection + RoPE =================
            with (
                tc.tile_pool(name="xtp", bufs=1) as xtp,
                tc.tile_pool(name="xvq", bufs=2) as xvqp,
                tc.tile_pool(name="wp", bufs=1) as wp,
                tc.tile_pool(name="raw", bufs=2) as rawp,
                tc.tile_pool(name="tmp", bufs=2) as tmpp,
                tc.tile_pool(name="psA", bufs=2, space="PSUM") as psA,
            ):
                xt8 = xtp.tile([128, MO, S], FP8, tag="xt8")
                wq = wp.tile([128, MO, R_LOCAL], FP8, tag="wq")
                wk = wp.tile([128, MO, R_LOCAL], FP8, tag="wk")
                wv = wp.tile([128, MO, R_LOCAL], F16, tag="wv")
                cs_t = wp.tile([128, S], F16, tag="cs")
                sc_t = wp.tile([128, S], F16, tag="sc")

                # warm up the PE / HAM while the first input DMAs land
                warmps = psA.tile([128, 128], F32, tag="psA")
                for _ in range(N_WARM):
                    nc.tensor.matmul(
                        warmps[:, :],
                        ones_t[:, :],
                        ones_t[:, :],
                        start=True,
                        stop=True,
                    )

                # DMA order: the first m-pairs' weights/x first (they gate the
                # first real matmul), then the rest in large contiguous spans.
                nc.sync.dma_start(out=wq[:, 0:4, :], in_=wqS_v[:, 0:4, :])
                nc.sync.dma_start(out=wk[:, 0:4, :], in_=wkS_v[:, 0:4, :])
                for mo in range(4):
                    nc.sync.dma_start(out=xt8[:, mo, :], in_=xT8_v[:, mo, :])
                for g4 in range(1, 4):
                    sl = slice(4 * g4, 4 * g4 + 4)
                    nc.sync.dma_start(out=wq[:, sl, :], in_=wqS_v[:, sl, :])
                    nc.sync.dma_start(out=wk[:, sl, :], in_=wkS_v[:, sl, :])
                    nc.sync.dma_start(out=xt8[:, sl, :], in_=xT8_v[:, sl, :])
                nc.sync.dma_start(out=cs_t[:, :], in_=cs_d[:, :])
                nc.sync.dma_start(out=sc_t[:, :], in_=sc_d[:, :])
                if causal:
                    nc.sync.dma_start(out=maskd[:, :], in_=maskd_d[:, :])
                nc.sync.dma_start(out=wv[:, :, :], in_=wvS_v[:, :, :])

                def rope(dst, raw):
                    """NeoX rotary: rows 0:64 = t*c - b*s ; rows 64:128 = t*s + b*c."""
                    ta = tmpp.tile([64, S], F16, tag="tmp")
                    tb = tmpp.tile([64, S], F16, tag="tmp")
                    nc.vector.tensor_mul(ta[:, :], raw[0:64, :], cs_t[0:64, :])
                    nc.vector.tensor_mul(tb[:, :], raw[64:128, :], cs_t[64:128, :])
                    nc.vector.tensor_sub(dst[0:64, :], ta[:, :], tb[:, :])
                    tc2 = tmpp.tile([64, S], F16, tag="tmp")
                    td = tmpp.tile([64, S], F16, tag="tmp")
                    nc.vector.tensor_mul(tc2[:, :], raw[0:64, :], sc_t[0:64, :])
                    nc.vector.tensor_mul(td[:, :], raw[64:128, :], sc_t[64:128, :])
                    nc.vector.tensor_add(dst[64:128, :], tc2[:, :], td[:, :])

                # head 0 q/k with the m-pair loop OUTER so the matmuls consume
                # xt8 m-blocks as the DMAs land (startup overlap).
                qp0 = psA.tile([128, S], F32, tag="psA")
                kp0 = psA.tile([128, S], F32, tag="psA")
                for mp in range(MP):
                    for c4 in range(4):
                        nc.tensor.matmul(
                            qp0[:, c4 * 512 : (c4 + 1) * 512],
                            wq[:, 2 * mp : 2 * mp + 2, 0:128],
                            xt8[:, 2 * mp : 2 * mp + 2, c4 * 512 : (c4 + 1) * 512],
                            start=(mp == 0),
                            stop=(mp == MP - 1),
                            perf_mode=DR,
                        )
                        nc.tensor.matmul(
                            kp0[:, c4 * 512 : (c4 + 1) * 512],
                            wk[:, 2 * mp : 2 * mp + 2, 0:128],
                            xt8[:, 2 * mp : 2 * mp + 2, c4 * 512 : (c4 + 1) * 512],
                            start=(mp == 0),
                            stop=(mp == MP - 1),
                            perf_mode=DR,
                        )
                q_raw = rawp.tile([128, S], F16, tag="raw")
                nc.scalar.copy(q_raw[:, :], qp0[:, :])
                rope(q_rot[:, 0, :], q_raw)
                k_raw = rawp.tile([128, S], F16, tag="raw")
                nc.scalar.copy(k_raw[:, :], kp0[:, :])
                rope(k_rot[:, 0, :], k_raw)

                def project(dst_raw, w_sb, h):
                    """q/k head projection -> fp16 raw [128, S] (xt8 resident)."""
                    ps = psA.tile([128, S], F32, tag="psA")
                    for c4 in range(4):
                        for mp in range(MP):
                            nc.tensor.matmul(
                                ps[:, c4 * 512 : (c4 + 1) * 512],
                                w_sb[:, 2 * mp : 2 * mp + 2, h * 128 : (h + 1) * 128],
                                xt8[:, 2 * mp : 2 * mp + 2, c4 * 512 : (c4 + 1) * 512],
                                start=(mp == 0),
                                stop=(mp == MP - 1),
                                perf_mode=DR,
                            )
                    nc.scalar.copy(dst_raw[:, :], ps[:, :])

                for h in range(1, HPC):
                    q_raw = rawp.tile([128, S], F16, tag="raw")
                    project(q_raw, wq, h)
                    rope(q_rot[:, h, :], q_raw)
                    k_raw = rawp.tile([128, S], F16, tag="raw")
                    project(k_raw, wk, h)
                    rope(k_rot[:, h, :], k_raw)

                # V projection (natural layout [s, r]) in fp16; x quarters are
                # streamed through a double-buffered pool (SBUF can't hold the
                # fp16 x alongside the fp8 copy).
                for sb4 in range(NSB // 4):
                    xq = xvqp.tile([128, MO, 512], F16, tag="xq")
                    nc.sync.dma_start(out=xq[:, :, :], in_=xQ16_v[:, sb4, :, :])
                    ps = psA.tile([128, S], F32, tag="psA")
                    for part in range(4):
                        for m in range(MO):
                            nc.tensor.matmul(
                                ps[:, part * 512 : part * 512 + 512],
                                xq[:, m, part * 128 : (part + 1) * 128],
                                wv[:, m, :],
                                start=(m == 0),
                                stop=(m == MO - 1),
                            )
                    nc.scalar.copy(v_sb[:, sb4 * 4 : sb4 * 4 + 4, :], ps[:, :])

            # ================= phase B: attention + out-projection =============
            with (
                tc.tile_pool(name="wop", bufs=1) as wop,
                tc.tile_pool(name="pp", bufs=6) as ppp,
                tc.tile_pool(name="pac", bufs=2) as pacp,
                tc.tile_pool(name="ao", bufs=2) as aop,
                tc.tile_pool(name="rcp", bufs=3) as rcp,
                tc.tile_pool(name="osb", bufs=3) as osbp,
                tc.tile_pool(name="mblk", bufs=4) as mblkp,
                tc.tile_pool(name="sp", bufs=2, space="PSUM") as spp,
                tc.tile_pool(name="acc", bufs=2, space="PSUM") as accp,
            ):
                wo = wop.tile([128, HPC, D_MODEL], F16, tag="wo")
                nc.sync.dma_start(out=wo[:, :, :], in_=woS_v[:, :, :])

                for qc in range(NQC):
                    q_lo = qc * 512
                    nki_here = (4 * qc + 4) if causal else NKI
                    aoT = aop.tile([128, HPC, 512], F16, tag="aoT")

                    # state for the lazily-emitted softmax denominator reduce
                    pending = []  # [(h, sav, pacc, split)]

                    def flush_sums():
                        if not pending:
                            return
                        h0, sav0, pacc0, split0 = pending.pop()
                        if split0:
                            # fold the right half of the wide accumulator in
                            nc.vector.tensor_add(
                                pacc0[:, 0:512], pacc0[:, 0:512], pacc0[:, 512:1024]
                            )
                        nc.tensor.matmul(
                            sav0[:, 0:512],
                            ones_t[:, :],
                            pacc0[:, 0:512],
                            start=True,
                            stop=True,
                        )
                        rc = rcp.tile([128, 512], F32, tag="rc")
                        nc.vector.reciprocal_approx_fast(rc[:, :], sav0[:, 0:512])
                        nc.vector.tensor_mul(
                            aoT[:, h0, :], sav0[:, 512:1024], rc[:, :]
                        )

                    for h in range(HPC):
                        sav = accp.tile([128, 1024], F32, tag="acc")
                        avp = sav[:, 512:1024]
                        pacc = pacp.tile([128, 1024], F16, tag="pac")
                        n_full = 4 * qc if causal else nki_here
                        # full key blocks, two at a time (one wide Exp each)
                        for kp in range(n_full // 2):
                            ki0 = 2 * kp
                            spb2 = spp.tile([128, 1024], F32, tag="sp")
                            pp2 = ppp.tile([128, 1024], F16, tag="pp")
                            for j in range(2):
                                ki = ki0 + j
                                nc.tensor.matmul(
                                    spb2[:, j * 512 : (j + 1) * 512],
                                    k_rot[:, h, ki * 128 : (ki + 1) * 128],
                                    q_rot[:, h, q_lo : q_lo + 512],
                                    start=True,
                                    stop=True,
                                )
                            nc.scalar.activation(
                                pp2[:, :],
                                spb2[:, :],
                                mybir.ActivationFunctionType.Exp,
                                scale=float(exp_scale),
                            )
                            if not causal:
                                for j in range(2):
                                    ki = ki0 + j
                                    mb = mblkp.tile([128, 512], F16, tag="mblk")
                                    nc.sync.dma_start(
                                        out=mb[:, :],
                                        in_=maskf_v[:, ki, q_lo : q_lo + 512],
                                    )
                                    nc.vector.tensor_mul(
                                        pp2[:, j * 512 : (j + 1) * 512],
                                        pp2[:, j * 512 : (j + 1) * 512],
                                        mb[:, :],
                                    )
                            if ki0 == 0:
                                nc.vector.tensor_copy(pacc[:, :], pp2[:, :])
                            else:
                                nc.vector.tensor_add(pacc[:, :], pacc[:, :], pp2[:, :])
                            for j in range(2):
                                ki = ki0 + j
                                nc.tensor.matmul(
                                    avp[:, :],
                                    v_sb[:, ki, h * 128 : (h + 1) * 128],
                                    pp2[:, j * 512 : (j + 1) * 512],
                                    start=(ki == 0),
                                    stop=(ki == nki_here - 1),
                                )
                            if kp == 0:
                                flush_sums()
                        # diagonal key blocks (causal only), two per Exp: the
                        # second block's scores are written right after the
                        # first's valid span so one gap-free Exp covers both.
                        for kd in range((nki_here - n_full) // 2):
                            kia = n_full + 2 * kd
                            q0a = 128 * (kia - 4 * qc)
                            q0b = q0a + 128
                            wb = 512 - q0b  # second block's valid width
                            spb2 = spp.tile([128, 1024], F32, tag="sp")
                            pp2 = ppp.tile([128, 1024], F16, tag="pp")
                            nc.tensor.matmul(
                                spb2[:, q0a:512],
                                k_rot[:, h, kia * 128 : (kia + 1) * 128],
                                q_rot[:, h, q_lo + q0a : q_lo + 512],
                                start=True,
                                stop=True,
                            )
                            nc.tensor.matmul(
                                spb2[:, 512 : 512 + wb],
                                k_rot[:, h, (kia + 1) * 128 : (kia + 2) * 128],
                                q_rot[:, h, q_lo + q0b : q_lo + 512],
                                start=True,
                                stop=True,
                            )
                            nc.scalar.activation(
                                pp2[:, q0a : 512 + wb],
                                spb2[:, q0a : 512 + wb],
                                mybir.ActivationFunctionType.Exp,
                                scale=float(exp_scale),
                            )
                            nc.vector.tensor_mul(
                                pp2[:, q0a : q0a + 128],
                                pp2[:, q0a : q0a + 128],
                                maskd[:, kia * 128 : (kia + 1) * 128],
                            )
                            nc.vector.tensor_mul(
                                pp2[:, 512:640],
                                pp2[:, 512:640],
                                maskd[:, (kia + 1) * 128 : (kia + 2) * 128],
                            )
                            if kia == 0:
                                nc.vector.tensor_copy(pacc[:, 0:512], pp2[:, 0:512])
                            else:
                                nc.vector.tensor_add(
                                    pacc[:, q0a:512], pacc[:, q0a:512], pp2[:, q0a:512]
                                )
                            nc.vector.tensor_add(
                                pacc[:, q0b:512],
                                pacc[:, q0b:512],
                                pp2[:, 512 : 512 + wb],
                            )
                            nc.tensor.matmul(
                                avp[:, q0a:512],
                                v_sb[:, kia, h * 128 : (h + 1) * 128],
                                pp2[:, q0a:512],
                                start=(kia == 0),
                                stop=False,
                            )
                            nc.tensor.matmul(
                                avp[:, q0b:512],
                                v_sb[:, kia + 1, h * 128 : (h + 1) * 128],
                                pp2[:, 512 : 512 + wb],
                                start=False,
                                stop=(kia + 1 == nki_here - 1),
                            )
                            if kd == 0:
                                flush_sums()
                        pending.append((h, sav, pacc, n_full > 0))

                    # out-projection for this query chunk (4 seq blocks);
                    # h outer over oc pairs so each aoT stationary load
                    # serves two matmuls.  The last head's softmax reduce is
                    # flushed after the first three heads' first matmuls so
                    # the PE stays busy while the Vector engine finishes it.
                    for sb_l in range(4):
                        sb = 4 * qc + sb_l
                        ob = osbp.tile([128, 2048], F16, tag="osb")
                        for oc2 in range(2):
                            op2 = accp.tile([128, 1024], F32, tag="acc")
                            for h in range(HPC):
                                if pending and h == HPC - 1:
                                    flush_sums()
                                lhsT = aoT[:, h, sb_l * 128 : (sb_l + 1) * 128]
                                nc.tensor.matmul(
                                    op2[:, 0:512],
                                    lhsT,
                                    wo[:, h, (2 * oc2) * 512 : (2 * oc2 + 1) * 512],
                                    start=(h == 0),
                                    stop=(h == HPC - 1),
                                )
                                nc.tensor.matmul(
                                    op2[:, 512:1024],
                                    lhsT,
                                    wo[:, h, (2 * oc2 + 1) * 512 : (2 * oc2 + 2) * 512],
                                    start=(h == 0),
                                    stop=(h == HPC - 1),
                                )
                            nc.vector.tensor_copy(
                                ob[:, oc2 * 1024 : (oc2 + 1) * 1024], op2[:, :]
                            )
                        nc.sync.dma_start(
                            out=out_d[sb * 128 : (sb + 1) * 128, :],
                            in_=ob[:, :],
                        )

    nc.finalize()
    return nc


def _bit_quantize_ternary(w: np.ndarray):
    """Returns (ternary {-1,0,1} float32 matrix, scale) matching the reference."""
    scale = np.maximum(np.mean(np.abs(w.astype(np.float32))), np.float32(1e-5))
    t = np.clip(np.round(w.astype(np.float32) / scale), -1.0, 1.0).astype(np.float32)
    return t, float(scale)


def _host_tables():
    """cos/sin stacked [128, S]: rows 0:64 cos, rows 64:128 sin."""
    inv_freq = 1.0 / (ROPE_BASE ** (np.arange(0, D_HEAD, 2, dtype=np.float32) / D_HEAD))
    pos = np.arange(SEQ, dtype=np.float32)
    ang = pos[:, None] * inv_freq[None, :]  # [S, 64]
    cs = np.empty((128, SEQ), dtype=NPF16)
    cs[0:64] = np.ascontiguousarray(np.cos(ang).T).astype(NPF16)
    cs[64:128] = np.ascontiguousarray(np.sin(ang).T).astype(NPF16)
    sc = np.empty((128, SEQ), dtype=NPF16)
    sc[0:64] = cs[64:128]
    sc[64:128] = cs[0:64]
    return cs, sc


def kernel(x, w_qkv, w_out, mask):
    global LAST_RESULT
    x = np.asarray(x, dtype=np.float32)
    w_qkv = np.asarray(w_qkv, dtype=np.float32)
    w_out = np.asarray(w_out, dtype=np.float32)
    mask = np.asarray(mask)

    tq, sq = _bit_quantize_ternary(w_qkv)
    to, so = _bit_quantize_ternary(w_out)
    exp_scale = (sq * sq) / float(np.sqrt(D_HEAD))
    c2 = np.float32(sq * so)

    m2 = (mask.reshape(SEQ, SEQ) != 0).astype(np.float32)
    causal = bool(np.array_equal(m2, np.tril(np.ones((SEQ, SEQ), np.float32))))

    cs, sc = _host_tables()
    if causal:
        maskd = np.empty((128, SEQ), dtype=NPF16)
        for ki in range(NKI):
            blk = m2[ki * 128 : (ki + 1) * 128, ki * 128 : (ki + 1) * 128]  # [q, k]
            maskd[:, ki * 128 : (ki + 1) * 128] = np.ascontiguousarray(blk.T).astype(
                NPF16
            )
    else:
        maskf = np.ascontiguousarray(m2.T).astype(NPF16)  # [kk, qq]

    key = (causal, float(exp_scale))
    if key not in _PROG_CACHE:
        _PROG_CACHE[key] = _build_program(causal, float(exp_scale))
    nc = _PROG_CACHE[key]

    def _swz_w(wT):  # [D_MODEL, R_LOCAL] -> [128, MO*R_LOCAL] partition-major
        return np.ascontiguousarray(
            wT.reshape(MO, 128, R_LOCAL).transpose(1, 0, 2).reshape(128, MO * R_LOCAL)
        )

    xT8 = [np.ascontiguousarray(x[b].T).astype(NPFP8) for b in range(BATCH)]
    xQ16 = [
        np.ascontiguousarray(
            x[b].T.reshape(MO, 128, 4, 512)
            .transpose(2, 1, 0, 3)
            .reshape(4 * 128, MO * 512)
        ).astype(NPF16)
        for b in range(BATCH)
    ]

    in_maps = []
    for c in range(N_CORES):
        b, g = divmod(c, 4)
        rows = slice(R_LOCAL * g, R_LOCAL * (g + 1))
        woT = to[:, rows].T  # [R_LOCAL, D_MODEL]
        woS = (
            woT.reshape(HPC, 128, D_MODEL)
            .transpose(1, 0, 2)
            .reshape(128, HPC * D_MODEL)
        )
        im = {
            "xT8": xT8[b],
            "xQ16": xQ16[b],
            "wqS": _swz_w(tq[0 * D_MODEL :][rows].T).astype(NPFP8),
            "wkS": _swz_w(tq[1 * D_MODEL :][rows].T).astype(NPFP8),
            "wvS": _swz_w(tq[2 * D_MODEL :][rows].T).astype(NPF16),
            "woS": np.ascontiguousarray(woS).astype(NPF16),
            "cossinT": cs,
            "sincosT": sc,
        }
        if causal:
            im["maskd"] = maskd
        else:
            im["maskf"] = maskf
        in_maps.append(im)

    do_trace = bool(PROFILE) and _enable_profiling()
    res = run_bass_kernel_spmd(nc, in_maps, list(range(N_CORES)), trace=do_trace)
    LAST_RESULT = res

    parts = [np.asarray(res.results[c]["out"]).astype(np.float32) for c in range(N_CORES)]
    out = np.stack(
        [
            parts[0] + parts[1] + parts[2] + parts[3],
            parts[4] + parts[5] + parts[6] + parts[7],
        ]
    )
    return (out * c2).astype(np.float32)
